# revision 38
# baseline (speedup 1.0000x reference)
"""Trainium2 Bass kernel for nn_Mlp_moe: dense patch-token MLP + top-1 gated
atom (expert) routing for 6 CLS task tokens.

Sharding over 8 NeuronCores:
  - Patch MLP: data-parallel over batch B=64 -> 8 batches (1568 patch tokens)
    per core. MLP weights replicated (SBUF-resident, bf16).
  - Atom/CLS part: hidden dim H=3072 sharded 8-way (384 per core); every core
    processes all 384 CLS tokens for all 5 atoms on its H-shard and emits a
    partial output summed on the host. Routing (gate logits/sigmoid/top-1
    masks) is computed on the host (it is O(B*6*D), negligible) and shipped
    as {0,1}/weight masks folded into the device compute.

Schedule (v2): the PE is warmed with dummy matmuls from program start (HAM
clock gate releases after ~3.4us of activity), while DMAs stream the first
working set (w1 piece 0 + x chunk 0) on need-ordered queues. w1 arrives in 8
pieces so GEMM1 starts as soon as the first 3 h-tiles + x0 land. Atom phases
run between chunk0's GEMM1 and GEMM2. Outputs stage through SBUF as bf16.

Device compute is bf16 (PSUM accumulation is fp32; erf-Gelu on ScalarE is
~exact); patch outputs are bf16, cls partials fp32.
"""

import numpy as np
import ml_dtypes

import concourse.bass as bass
import concourse.bacc as bacc
import concourse.mybir as mybir
from concourse import tile
from concourse.bass_utils import run_bass_kernel_spmd

NCORES = 8
B, NCLS, P, D, H = 64, 6, 196, 768, 3072
NA = 5
HSH = H // NCORES            # 384: per-core atom hidden shard
BPC = B // NCORES            # 8 batches per core
TPC = BPC * P                # 1568 patch tokens per core
NT = B * NCLS                # 384 cls tokens
DT = D // 128                # 6 d-tiles
HT = H // 128                # 24 h-tiles
HLT = NA * HSH // 128        # 15 atom h-shard tiles (a-major, 3 per atom)
KPA = HSH // 128             # 3 h-shard tiles per atom
CW = 392
NCH = 4
CHUNKS = [(i * CW, CW) for i in range(NCH)]
NW1P = 8                     # w1 DMA pieces (3 h-tiles each)
HPP = HT // NW1P             # 3 h-tiles per piece
NWARM = 30                   # PE warm-up dummy matmuls

LEFT_KEYS = np.array([3, 4, 8, 9, 13, 14], dtype=np.int64)
RIGHT_KEYS = np.array([15, 20, 16, 21, 17, 22], dtype=np.int64)

BF16 = mybir.dt.bfloat16
F32 = mybir.dt.float32
AF = mybir.ActivationFunctionType

_CACHE = {}
LAST_RESULTS = None  # BassKernelResults of the most recent run (for profiling)


def _build_program(goff, dranges):
    """goff: 6 cumulative offsets of the 5 src-atom token groups (cls tokens
    are host-permuted by (src, dst) so each atom's tokens are a contiguous
    column range). dranges[a]: list of (start, end) column ranges whose
    tokens route their output through atom a.
    """
    nc = bacc.Bacc(None, target_bir_lowering=False, debug=False,
                   num_devices=NCORES)

    # partition-major packed inputs (see host layouts in kernel())
    xT_d = nc.dram_tensor("xT", [128, NCH * DT * CW], BF16,
                          kind="ExternalInput")
    w1T_d = nc.dram_tensor("w1T", [128, NW1P, DT * HPP * 128], BF16,
                           kind="ExternalInput")
    b1T_d = nc.dram_tensor("b1T", [128, HT], F32, kind="ExternalInput")
    w2T_d = nc.dram_tensor("w2T", [128, HT * D], BF16, kind="ExternalInput")
    clsT_d = nc.dram_tensor("clsT", [128, DT * NT], BF16,
                            kind="ExternalInput")
    ainT_d = nc.dram_tensor("ainT", [DT, 128, NA * HSH], BF16,
                            kind="ExternalInput")
    ainbT_d = nc.dram_tensor("ainbT", [128, HLT], F32, kind="ExternalInput")
    aoutT_d = nc.dram_tensor("aoutT", [NA, 128, KPA * D], BF16,
                             kind="ExternalInput")
    wrep_d = nc.dram_tensor("wrep", [128, NT], BF16, kind="ExternalInput")
    poutT_d = nc.dram_tensor("poutT", [DT, 128, TPC], BF16,
                             kind="ExternalOutput")
    cpartT_d = nc.dram_tensor("cpartT", [DT, 128, NT], BF16,
                              kind="ExternalOutput")

    with tile.TileContext(nc) as tc:
        with (
            tc.tile_pool(name="w", bufs=1) as wp,
            tc.tile_pool(name="gat", bufs=1) as gp,
            tc.tile_pool(name="hida", bufs=1) as hp,
            tc.tile_pool(name="xin", bufs=2) as xp,
            tc.tile_pool(name="g1", bufs=48) as g1p,
            tc.tile_pool(name="ostg", bufs=4) as op,
            tc.tile_pool(name="ps", bufs=7, space="PSUM") as pp,
            tc.tile_pool(name="psw", bufs=1, space="PSUM") as pwp,
        ):
            # ---- PE warm-up: dummy matmuls from program start ----
            # The HAM clock gate holds the PE at 1.2 GHz until ~3.4us of
            # sustained activity; these dummies run while the first DMAs
            # stream in so the real matmuls start at 2.4 GHz.
            wdum = wp.tile([128, 512], BF16, tag="wdum", name="wdum")
            nc.vector.memset(wdum[:], 0.03125)
            pdum = pwp.tile([128, 256], F32, tag="pdum", name="pdum")
            for _ in range(NWARM):
                nc.tensor.matmul(pdum[:], wdum[:, :128], wdum[:, :256],
                                 start=True, stop=True)

            # ---- DMA issues: global need order striped over the 3 DMA
            # queues (sync/scalar/gpsimd) so the heads of all queues are
            # always the next-needed tensors and the shared DGE engine pool
            # serves the critical path first.
            def load_x(ci):
                xa = xp.tile([128, DT * CW], BF16, tag="x", name="x")
                nc.sync.dma_start(
                    xa[:], xT_d[:, ci * DT * CW:(ci + 1) * DT * CW])
                return xa

            w1T = [wp.tile([128, DT * HPP * 128], BF16, tag=f"w1{q}",
                           name=f"w1{q}") for q in range(NW1P)]
            b1T = wp.tile([128, HT], F32, tag="b1", name="b1")
            clsT = wp.tile([128, DT * NT], BF16, tag="cls", name="cls")
            w2T = wp.tile([128, HT * D], BF16, tag="w2", name="w2")
            ainbT = wp.tile([128, HLT], F32, tag="ainb", name="ainb")
            ainT = [wp.tile([128, NA * HSH], BF16, tag=f"ain{d}",
                            name=f"ain{d}") for d in range(DT)]
            wrep = wp.tile([128, NT], BF16, tag="wr", name="wr")
            aoutT = [wp.tile([128, KPA * D], BF16, tag=f"ao{a}",
                             name=f"ao{a}") for a in range(NA)]

            # wave 1: chunk0 GEMM1 working set. x0 and every w1 piece are
            # split in d-thirds round-robined over all three queues, so the
            # shared DGE pool delivers them in exact need order at full
            # aggregate bandwidth (piece k lands before the matmuls for
            # piece k-1 complete).
            # NOTE: an engine's next dma_start blocks until its previous
            # transfer completes, so ScalarE (which must run the gelus from
            # ~16us on) gets only the 3 earliest DMAs; sync/gpsimd (no
            # compute duties) carry everything else.
            xa0 = xp.tile([128, DT * CW], BF16, tag="x", name="x")
            nc.sync.dma_start(xa0[:, :3 * CW], xT_d[:, :3 * CW])
            nc.gpsimd.dma_start(xa0[:, 3 * CW:DT * CW],
                                xT_d[:, 3 * CW:DT * CW])
            xs_pre = [xa0]
            nc.scalar.dma_start(b1T[:], b1T_d[:])
            w1c = DT * HPP * 128
            nc.scalar.dma_start(w1T[0][:, :w1c // 3], w1T_d[:, 0, :w1c // 3])
            nc.sync.dma_start(w1T[0][:, w1c // 3:2 * w1c // 3],
                              w1T_d[:, 0, w1c // 3:2 * w1c // 3])
            nc.gpsimd.dma_start(w1T[0][:, 2 * w1c // 3:],
                                w1T_d[:, 0, 2 * w1c // 3:])
            for p in range(1, NW1P):
                h1, h2 = (nc.sync, nc.gpsimd) if p % 2 else \
                    (nc.gpsimd, nc.sync)
                h1.dma_start(w1T[p][:, :w1c // 2], w1T_d[:, p, :w1c // 2])
                h2.dma_start(w1T[p][:, w1c // 2:], w1T_d[:, p, w1c // 2:])
            # wave 2: chunk1 x (on the otherwise-idle scalar queue, so it
            # lands long before G1c1 without displacing the w1 pieces) +
            # atom-in tensors
            xa1 = xp.tile([128, DT * CW], BF16, tag="x", name="x")
            nc.scalar.dma_start(xa1[:], xT_d[:, DT * CW:2 * DT * CW])
            xs_pre.append(xa1)
            nc.gpsimd.dma_start(ainT[0][:], ainT_d[0])
            nc.sync.dma_start(clsT[:], clsT_d[:])
            nc.gpsimd.dma_start(ainT[1][:], ainT_d[1])
            nc.sync.dma_start(ainT[2][:], ainT_d[2])
            nc.gpsimd.dma_start(ainT[3][:], ainT_d[3])
            nc.sync.dma_start(ainT[4][:], ainT_d[4])
            nc.gpsimd.dma_start(ainT[5][:], ainT_d[5])
            nc.sync.dma_start(ainbT[:], ainbT_d[:])
            # wave 3: w2 + gate weights + atom-out tensors
            nc.gpsimd.dma_start(w2T[:, :12 * D], w2T_d[:, :12 * D])
            nc.sync.dma_start(w2T[:, 12 * D:], w2T_d[:, 12 * D:])
            nc.gpsimd.dma_start(wrep[:], wrep_d[:])
            nc.sync.dma_start(aoutT[0][:], aoutT_d[0])
            nc.gpsimd.dma_start(aoutT[1][:], aoutT_d[1])
            nc.sync.dma_start(aoutT[2][:], aoutT_d[2])
            nc.gpsimd.dma_start(aoutT[3][:], aoutT_d[3])
            nc.sync.dma_start(aoutT[4][:], aoutT_d[4])

            # ---- patch GEMM1 for one chunk (piece-gated on first chunk) ----
            def patch_g1(ci, xa):
                c0, cw = CHUNKS[ci]
                g1s = []
                for h in range(HT):
                    ps = pp.tile([128, 512], F32, tag="ps", name="ps")
                    q, hh = divmod(h, HPP)
                    for d in range(DT):
                        nc.tensor.matmul(
                            ps[:, :cw],
                            w1T[q][:, d * HPP * 128 + hh * 128:
                                   d * HPP * 128 + (hh + 1) * 128],
                            xa[:, d * CW:d * CW + cw],
                            start=(d == 0), stop=(d == DT - 1))
                    g1 = g1p.tile([128, CW], BF16, tag="g1", name="g1")
                    nc.scalar.activation(g1[:, :cw], ps[:, :cw], AF.Gelu,
                                         bias=b1T[:, h:h + 1])
                    g1s.append(g1)
                return g1s

            def patch_g2(ci, g1s):
                c0, cw = CHUNKS[ci]
                for dp in range(DT):
                    ps = pp.tile([128, 512], F32, tag="ps", name="ps")
                    for h in range(HT):
                        nc.tensor.matmul(
                            ps[:, :cw],
                            w2T[:, h * D + dp * 128:h * D + (dp + 1) * 128],
                            g1s[h][:, :cw],
                            start=(h == 0), stop=(h == HT - 1))
                    stg = op.tile([128, CW], BF16, tag="ostg", name="ostg")
                    nc.vector.tensor_copy(stg[:, :cw], ps[:, :cw])
                    nc.gpsimd.dma_start(poutT_d[dp][:, c0:c0 + cw],
                                        stg[:, :cw])

            g1s_c0 = patch_g1(0, xs_pre[0])
            g1s_c1 = patch_g1(1, xs_pre[1])

            # ---- phase A: grouped atom in-GEMM + gelu ----
            # cls tokens are host-permuted by src atom: group s occupies
            # columns [goff[s], goff[s+1]), so each token's hidden state is
            # computed only for its routed atom (1/5 the FLOPs of the dense
            # all-atom form).
            Gk = [gp.tile([128, NT], BF16, tag=f"g{k}", name=f"g{k}")
                  for k in range(KPA)]
            for s in range(NA):
                o0, o1 = goff[s], goff[s + 1]
                ns = o1 - o0
                if ns == 0:
                    continue
                for k in range(KPA):
                    ps = pp.tile([128, 512], F32, tag="ps", name="ps")
                    c0 = s * HSH + k * 128
                    for d in range(DT):
                        nc.tensor.matmul(
                            ps[:, :ns],
                            ainT[d][:, c0:c0 + 128],
                            clsT[:, d * NT + o0:d * NT + o1],
                            start=(d == 0), stop=(d == DT - 1))
                    nc.scalar.activation(Gk[k][:, o0:o1], ps[:, :ns],
                                         AF.Gelu,
                                         bias=ainbT[:, s * KPA + k:
                                                    s * KPA + k + 1])

            # ---- phase B: scale hidden by the gate weight (DVE) ----
            Hk = []
            for k in range(KPA):
                h = hp.tile([128, NT], BF16, tag=f"hid{k}", name=f"hid{k}")
                nc.vector.tensor_mul(h[:], Gk[k][:], wrep[:])
                Hk.append(h)

            patch_g2(0, g1s_c0)

            # ---- atom out-GEMM, grouped by dst atom ----
            # Each column range in dranges[a] holds tokens routed to atom a;
            # each range accumulates over the KPA h-shard tiles only.
            # PSUM zero regions are whole banks: the FIRST matmul into the
            # tile carries start=True (lazily zeroing the bank); every other
            # matmul accumulates — first touch of a pending byte zeroes it.
            nmm_out = sum(KPA * len(dranges[a]) for a in range(NA))
            for dp in range(DT):
                ps = pp.tile([128, 512], F32, tag="ps", name="ps")
                n = 0
                for a in range(NA):
                    for k in range(KPA):
                        for (r0, r1) in dranges[a]:
                            nc.tensor.matmul(
                                ps[:, r0:r1],
                                aoutT[a][:, k * D + dp * 128:
                                         k * D + (dp + 1) * 128],
                                Hk[k][:, r0:r1],
                                start=(n == 0), stop=(n == nmm_out - 1),
                                skip_group_check=True)
                            n += 1
                stg = op.tile([128, CW], BF16, tag="cstg", name="cstg")
                nc.vector.tensor_copy(stg[:, :NT], ps[:, :NT])
                nc.gpsimd.dma_start(cpartT_d[dp], stg[:, :NT])

            # ---- patch chunks 1..3 ----
            patch_g2(1, g1s_c1)
            xs_pre.append(load_x(2))
            patch_g2(2, patch_g1(2, xs_pre[2]))
            xs_pre.append(load_x(3))
            patch_g2(3, patch_g1(3, xs_pre[3]))

    nc.compile()
    return nc


def _sigmoid(x):
    out = np.empty_like(x)
    pos = x >= 0
    out[pos] = 1.0 / (1.0 + np.exp(-x[pos]))
    ex = np.exp(x[~pos])
    out[~pos] = ex / (1.0 + ex)
    return out


def kernel(x, patch_w1, patch_b1, patch_w2, patch_b2, gate_delta,
           atom_in_w, atom_in_b, atom_out_w, atom_out_b):
    x = np.asarray(x, dtype=np.float32)
    patch_w1 = np.asarray(patch_w1, dtype=np.float32)
    patch_b1 = np.asarray(patch_b1, dtype=np.float32)
    patch_w2 = np.asarray(patch_w2, dtype=np.float32)
    patch_b2 = np.asarray(patch_b2, dtype=np.float32)
    gate_delta = np.asarray(gate_delta, dtype=np.float32)
    atom_in_w = np.asarray(atom_in_w, dtype=np.float32)
    atom_in_b = np.asarray(atom_in_b, dtype=np.float32)
    atom_out_w = np.asarray(atom_out_w, dtype=np.float32)
    atom_out_b = np.asarray(atom_out_b, dtype=np.float32)

    bf = ml_dtypes.bfloat16

    # ---- host routing (tiny) ----
    cls3 = x[:, :NCLS, :]                                   # [B, 6, D]
    logits = np.einsum("bnd,nd->bn", cls3, gate_delta)      # [B, 6] f32
    choose_left = logits >= 0
    p_left = _sigmoid(logits)
    wgt = np.where(choose_left, p_left, 1.0 - p_left).astype(np.float32)
    keys = np.where(choose_left, LEFT_KEYS[None, :], RIGHT_KEYS[None, :])
    src = (keys // NA).reshape(-1)                          # [384]
    dst = (keys % NA).reshape(-1)
    wflat = wgt.reshape(-1)                                 # [384]

    # permute cls tokens by (src, dst) so each src atom's tokens are
    # contiguous and each dst atom's tokens are a few contiguous ranges
    order = np.lexsort((dst, src))
    inv_order = np.argsort(order)
    src_p, dst_p, wflat_p = src[order], dst[order], wflat[order]
    goff = tuple(int(np.searchsorted(src_p, s)) for s in range(NA + 1))
    dranges = []
    for a in range(NA):
        idx = np.flatnonzero(dst_p == a)
        ranges = []
        if idx.size:
            brk = np.flatnonzero(np.diff(idx) > 1)
            starts = np.concatenate(([0], brk + 1))
            ends = np.concatenate((brk, [idx.size - 1]))
            ranges = [(int(idx[s]), int(idx[e]) + 1)
                      for s, e in zip(starts, ends)]
        dranges.append(tuple(ranges))
    dranges = tuple(dranges)

    wrep_rep = np.ascontiguousarray(
        np.broadcast_to(wflat_p.reshape(1, NT), (128, NT))).astype(bf)

    # ---- replicated tensors (partition-major packed) ----
    # clsT[p, d*NT + t] = cls_permuted[t, d*128+p]
    clsT = np.ascontiguousarray(
        cls3.reshape(NT, D)[order].reshape(NT, DT, 128).transpose(2, 1, 0)
    ).reshape(128, DT * NT).astype(bf)
    # w1T[p, q, d*384 + hh*128 + m] = patch_w1[(q*3+hh)*128+m, d*128+p]
    w1T = np.ascontiguousarray(
        patch_w1.reshape(NW1P, HPP, 128, DT, 128).transpose(4, 0, 3, 1, 2)
    ).reshape(128, NW1P, DT * HPP * 128).astype(bf)
    b1T = np.ascontiguousarray(patch_b1.reshape(HT, 128).T)
    # w2T[p, h*D + dp*128 + m] = patch_w2[dp*128+m, h*128+p]
    w2T = np.ascontiguousarray(
        patch_w2.reshape(DT, 128, HT, 128).transpose(3, 2, 0, 1)
    ).reshape(128, HT * D).astype(bf)

    # ---- per-core tensors ----
    patch = x[:, NCLS:, :].reshape(NCORES, TPC, D)
    # xT[p, ci*DT*CW + d*CW + t] = patch[c][ci*CW+t, d*128+p]
    xT_all = np.ascontiguousarray(
        patch.reshape(NCORES, NCH, CW, DT, 128).transpose(0, 4, 1, 3, 2)
    ).reshape(NCORES, 128, NCH * DT * CW).astype(bf)

    ainT_all, ainbT_all, aoutT_all = [], [], []
    for c in range(NCORES):
        hsl = slice(HSH * c, HSH * (c + 1))
        # ainT[d, p, a*HSH + k*128 + m] = atom_in_w[a, hsl0 + k*128+m, d*128+p]
        ainT = np.ascontiguousarray(
            atom_in_w[:, hsl, :].reshape(NA, KPA, 128, DT, 128)
            .transpose(3, 4, 0, 1, 2)).reshape(DT, 128, NA * HSH).astype(bf)
        ainT_all.append(ainT)
        ainbT_all.append(np.ascontiguousarray(
            atom_in_b[:, hsl].reshape(HLT, 128).T))
        # aoutT[a, p, k*D + dp*128 + m] = atom_out_w[a, dp*128+m, hsl0+k*128+p]
        aoutT = np.ascontiguousarray(
            atom_out_w[:, :, hsl].reshape(NA, DT, 128, KPA, 128)
            .transpose(0, 4, 3, 1, 2)).reshape(NA, 128, KPA * D).astype(bf)
        aoutT_all.append(aoutT)

    in_maps = []
    for c in range(NCORES):
        in_maps.append({
            "xT": xT_all[c], "w1T": w1T, "b1T": b1T, "w2T": w2T,
            "clsT": clsT, "ainT": ainT_all[c], "ainbT": ainbT_all[c],
            "aoutT": aoutT_all[c], "wrep": wrep_rep,
        })

    key = (goff, dranges)
    nc = _CACHE.get(key)
    if nc is None:
        nc = _build_program(goff, dranges)
        _CACHE[key] = nc

    res = run_bass_kernel_spmd(nc, in_maps, core_ids=list(range(NCORES)))
    global LAST_RESULTS
    LAST_RESULTS = res

    # ---- host gather ----
    patch_out = np.empty((B, P, D), dtype=np.float32)
    for c in range(NCORES):
        poutT = res.results[c]["poutT"].reshape(D, TPC).astype(np.float32)
        patch_out[BPC * c:BPC * (c + 1)] = (
            poutT.T + patch_b2[None, :]).reshape(BPC, P, D)

    cpart = np.zeros((D, NT), dtype=np.float32)
    for c in range(NCORES):
        cpart += res.results[c]["cpartT"].reshape(D, NT).astype(np.float32)
    cls_out = cpart.T[inv_order] + wflat[:, None] * atom_out_b[dst, :]
    cls_out = cls_out.reshape(B, NCLS, D)

    return np.concatenate([cls_out, patch_out], axis=1)


# revision 40
# speedup vs baseline: 1.0154x; 1.0154x over previous
"""Trainium2 Bass kernel for nn_Mlp_moe: dense patch-token MLP + top-1 gated
atom (expert) routing for 6 CLS task tokens.

Sharding over 8 NeuronCores:
  - Patch MLP: data-parallel over batch B=64 -> 8 batches (1568 patch tokens)
    per core. MLP weights replicated (SBUF-resident, bf16).
  - Atom/CLS part: hidden dim H=3072 sharded 8-way (384 per core); every core
    processes all 384 CLS tokens for all 5 atoms on its H-shard and emits a
    partial output summed on the host. Routing (gate logits/sigmoid/top-1
    masks) is computed on the host (it is O(B*6*D), negligible) and shipped
    as {0,1}/weight masks folded into the device compute.

Schedule (v2): the PE is warmed with dummy matmuls from program start (HAM
clock gate releases after ~3.4us of activity), while DMAs stream the first
working set (w1 piece 0 + x chunk 0) on need-ordered queues. w1 arrives in 8
pieces so GEMM1 starts as soon as the first 3 h-tiles + x0 land. Atom phases
run between chunk0's GEMM1 and GEMM2. Outputs stage through SBUF as bf16.

Device compute is bf16 (PSUM accumulation is fp32; erf-Gelu on ScalarE is
~exact); patch outputs are bf16, cls partials fp32.
"""

import numpy as np
import ml_dtypes

import concourse.bass as bass
import concourse.bacc as bacc
import concourse.mybir as mybir
from concourse import tile
from concourse.bass_utils import run_bass_kernel_spmd

NCORES = 8
B, NCLS, P, D, H = 64, 6, 196, 768, 3072
NA = 5
HSH = H // NCORES            # 384: per-core atom hidden shard
BPC = B // NCORES            # 8 batches per core
TPC = BPC * P                # 1568 patch tokens per core
NT = B * NCLS                # 384 cls tokens
DT = D // 128                # 6 d-tiles
HT = H // 128                # 24 h-tiles
HLT = NA * HSH // 128        # 15 atom h-shard tiles (a-major, 3 per atom)
KPA = HSH // 128             # 3 h-shard tiles per atom
CW = 392
NCH = 4
CHUNKS = [(i * CW, CW) for i in range(NCH)]
NW1P = 8                     # w1 DMA pieces (3 h-tiles each)
HPP = HT // NW1P             # 3 h-tiles per piece
NWARM = 52                   # PE warm-up dummy matmuls

LEFT_KEYS = np.array([3, 4, 8, 9, 13, 14], dtype=np.int64)
RIGHT_KEYS = np.array([15, 20, 16, 21, 17, 22], dtype=np.int64)

BF16 = mybir.dt.bfloat16
F32 = mybir.dt.float32
AF = mybir.ActivationFunctionType

_CACHE = {}
LAST_RESULTS = None  # BassKernelResults of the most recent run (for profiling)


def _build_program(goff, dranges):
    """goff: 6 cumulative offsets of the 5 src-atom token groups (cls tokens
    are host-permuted by (src, dst) so each atom's tokens are a contiguous
    column range). dranges[a]: list of (start, end) column ranges whose
    tokens route their output through atom a.
    """
    nc = bacc.Bacc(None, target_bir_lowering=False, debug=False,
                   num_devices=NCORES)

    # partition-major packed inputs (see host layouts in kernel())
    xT_d = nc.dram_tensor("xT", [128, NCH * DT * CW], BF16,
                          kind="ExternalInput")
    w1T_d = nc.dram_tensor("w1T", [128, NW1P, DT * HPP * 128], BF16,
                           kind="ExternalInput")
    b1T_d = nc.dram_tensor("b1T", [128, HT], F32, kind="ExternalInput")
    w2T_d = nc.dram_tensor("w2T", [128, HT * D], BF16, kind="ExternalInput")
    clsT_d = nc.dram_tensor("clsT", [128, DT * NT], BF16,
                            kind="ExternalInput")
    ainT_d = nc.dram_tensor("ainT", [DT, 128, NA * HSH], BF16,
                            kind="ExternalInput")
    ainbT_d = nc.dram_tensor("ainbT", [128, HLT], F32, kind="ExternalInput")
    aoutT_d = nc.dram_tensor("aoutT", [NA, 128, KPA * D], BF16,
                             kind="ExternalInput")
    wrep_d = nc.dram_tensor("wrep", [128, NT], BF16, kind="ExternalInput")
    poutT_d = nc.dram_tensor("poutT", [DT, 128, TPC], BF16,
                             kind="ExternalOutput")
    cpartT_d = nc.dram_tensor("cpartT", [DT, 128, NT], BF16,
                              kind="ExternalOutput")

    with tile.TileContext(nc) as tc:
        with (
            tc.tile_pool(name="w", bufs=1) as wp,
            tc.tile_pool(name="gat", bufs=1) as gp,
            tc.tile_pool(name="hida", bufs=1) as hp,
            tc.tile_pool(name="xin", bufs=2) as xp,
            tc.tile_pool(name="g1", bufs=48) as g1p,
            tc.tile_pool(name="ostg", bufs=4) as op,
            tc.tile_pool(name="ps", bufs=7, space="PSUM") as pp,
            tc.tile_pool(name="psw", bufs=1, space="PSUM") as pwp,
        ):
            # ---- PE warm-up: dummy matmuls from program start ----
            # The HAM clock gate holds the PE at 1.2 GHz until ~3.4us of
            # sustained activity; these dummies run while the first DMAs
            # stream in so the real matmuls start at 2.4 GHz.
            wdum = wp.tile([128, 512], BF16, tag="wdum", name="wdum")
            nc.vector.memset(wdum[:], 0.03125)
            pdum = pwp.tile([128, 256], F32, tag="pdum", name="pdum")
            for _ in range(NWARM):
                nc.tensor.matmul(pdum[:], wdum[:, :128], wdum[:, :256],
                                 start=True, stop=True)

            # ---- DMA issues: global need order striped over the 3 DMA
            # queues (sync/scalar/gpsimd) so the heads of all queues are
            # always the next-needed tensors and the shared DGE engine pool
            # serves the critical path first.
            def load_x(ci):
                xa = xp.tile([128, DT * CW], BF16, tag="x", name="x")
                nc.sync.dma_start(
                    xa[:], xT_d[:, ci * DT * CW:(ci + 1) * DT * CW])
                return xa

            w1T = [wp.tile([128, DT * HPP * 128], BF16, tag=f"w1{q}",
                           name=f"w1{q}") for q in range(NW1P)]
            b1T = wp.tile([128, HT], F32, tag="b1", name="b1")
            clsT = wp.tile([128, DT * NT], BF16, tag="cls", name="cls")
            w2T = wp.tile([128, HT * D], BF16, tag="w2", name="w2")
            ainbT = wp.tile([128, HLT], F32, tag="ainb", name="ainb")
            ainT = [wp.tile([128, NA * HSH], BF16, tag=f"ain{d}",
                            name=f"ain{d}") for d in range(DT)]
            wrep = wp.tile([128, NT], BF16, tag="wr", name="wr")
            aoutT = [wp.tile([128, KPA * D], BF16, tag=f"ao{a}",
                             name=f"ao{a}") for a in range(NA)]

            # wave 1: chunk0 GEMM1 working set. x0 and every w1 piece are
            # split in d-thirds round-robined over all three queues, so the
            # shared DGE pool delivers them in exact need order at full
            # aggregate bandwidth (piece k lands before the matmuls for
            # piece k-1 complete).
            # NOTE: an engine's next dma_start blocks until its previous
            # transfer completes, so ScalarE (which must run the gelus from
            # ~16us on) gets only the 3 earliest DMAs; sync/gpsimd (no
            # compute duties) carry everything else.
            xa0 = xp.tile([128, DT * CW], BF16, tag="x", name="x")
            qs = [nc.sync, nc.scalar, nc.gpsimd]
            for i, q in enumerate(qs):
                q.dma_start(xa0[:, i * 2 * CW:(i + 1) * 2 * CW],
                            xT_d[:, i * 2 * CW:(i + 1) * 2 * CW])
            xs_pre = [xa0]
            nc.scalar.dma_start(b1T[:], b1T_d[:])
            w1c = DT * HPP * 128
            nc.scalar.dma_start(w1T[0][:, :w1c // 3], w1T_d[:, 0, :w1c // 3])
            nc.sync.dma_start(w1T[0][:, w1c // 3:2 * w1c // 3],
                              w1T_d[:, 0, w1c // 3:2 * w1c // 3])
            nc.gpsimd.dma_start(w1T[0][:, 2 * w1c // 3:],
                                w1T_d[:, 0, 2 * w1c // 3:])
            for p in range(1, NW1P):
                h1, h2 = (nc.sync, nc.gpsimd) if p % 2 else \
                    (nc.gpsimd, nc.sync)
                h1.dma_start(w1T[p][:, :w1c // 2], w1T_d[:, p, :w1c // 2])
                h2.dma_start(w1T[p][:, w1c // 2:], w1T_d[:, p, w1c // 2:])
            # wave 2: chunk1 x (halves on both queues so it lands right as
            # G1c0's matmuls finish) + atom-in tensors
            xa1 = xp.tile([128, DT * CW], BF16, tag="x", name="x")
            nc.sync.dma_start(xa1[:, :3 * CW],
                              xT_d[:, DT * CW:DT * CW + 3 * CW])
            nc.gpsimd.dma_start(xa1[:, 3 * CW:],
                                xT_d[:, DT * CW + 3 * CW:2 * DT * CW])
            xs_pre.append(xa1)
            nc.gpsimd.dma_start(ainT[0][:], ainT_d[0])
            nc.sync.dma_start(clsT[:], clsT_d[:])
            nc.gpsimd.dma_start(ainT[1][:], ainT_d[1])
            nc.sync.dma_start(ainT[2][:], ainT_d[2])
            nc.gpsimd.dma_start(ainT[3][:], ainT_d[3])
            nc.sync.dma_start(ainT[4][:], ainT_d[4])
            nc.gpsimd.dma_start(ainT[5][:], ainT_d[5])
            nc.sync.dma_start(ainbT[:], ainbT_d[:])
            # wave 3: w2 + gate weights + atom-out tensors
            nc.gpsimd.dma_start(w2T[:, :12 * D], w2T_d[:, :12 * D])
            nc.sync.dma_start(w2T[:, 12 * D:], w2T_d[:, 12 * D:])
            nc.gpsimd.dma_start(wrep[:], wrep_d[:])
            nc.sync.dma_start(aoutT[0][:], aoutT_d[0])
            nc.gpsimd.dma_start(aoutT[1][:], aoutT_d[1])
            nc.sync.dma_start(aoutT[2][:], aoutT_d[2])
            nc.gpsimd.dma_start(aoutT[3][:], aoutT_d[3])
            nc.sync.dma_start(aoutT[4][:], aoutT_d[4])

            # ---- patch GEMM1 for one chunk (piece-gated on first chunk) ----
            def patch_g1(ci, xa):
                c0, cw = CHUNKS[ci]
                g1s = []
                for h in range(HT):
                    ps = pp.tile([128, 512], F32, tag="ps", name="ps")
                    q, hh = divmod(h, HPP)
                    for d in range(DT):
                        nc.tensor.matmul(
                            ps[:, :cw],
                            w1T[q][:, d * HPP * 128 + hh * 128:
                                   d * HPP * 128 + (hh + 1) * 128],
                            xa[:, d * CW:d * CW + cw],
                            start=(d == 0), stop=(d == DT - 1))
                    g1 = g1p.tile([128, CW], BF16, tag="g1", name="g1")
                    nc.scalar.activation(g1[:, :cw], ps[:, :cw], AF.Gelu,
                                         bias=b1T[:, h:h + 1])
                    g1s.append(g1)
                return g1s

            def patch_g2(ci, g1s):
                c0, cw = CHUNKS[ci]
                for dp in range(DT):
                    ps = pp.tile([128, 512], F32, tag="ps", name="ps")
                    for h in range(HT):
                        nc.tensor.matmul(
                            ps[:, :cw],
                            w2T[:, h * D + dp * 128:h * D + (dp + 1) * 128],
                            g1s[h][:, :cw],
                            start=(h == 0), stop=(h == HT - 1))
                    stg = op.tile([128, CW], BF16, tag="ostg", name="ostg")
                    nc.vector.tensor_copy(stg[:, :cw], ps[:, :cw])
                    nc.gpsimd.dma_start(poutT_d[dp][:, c0:c0 + cw],
                                        stg[:, :cw])

            g1s_c0 = patch_g1(0, xs_pre[0])
            g1s_c1 = patch_g1(1, xs_pre[1])

            # ---- phase A: grouped atom in-GEMM + gelu ----
            # cls tokens are host-permuted by src atom: group s occupies
            # columns [goff[s], goff[s+1]), so each token's hidden state is
            # computed only for its routed atom (1/5 the FLOPs of the dense
            # all-atom form).
            Gk = [gp.tile([128, NT], BF16, tag=f"g{k}", name=f"g{k}")
                  for k in range(KPA)]
            for s in range(NA):
                o0, o1 = goff[s], goff[s + 1]
                ns = o1 - o0
                if ns == 0:
                    continue
                for k in range(KPA):
                    ps = pp.tile([128, 512], F32, tag="ps", name="ps")
                    c0 = s * HSH + k * 128
                    for d in range(DT):
                        nc.tensor.matmul(
                            ps[:, :ns],
                            ainT[d][:, c0:c0 + 128],
                            clsT[:, d * NT + o0:d * NT + o1],
                            start=(d == 0), stop=(d == DT - 1))
                    nc.scalar.activation(Gk[k][:, o0:o1], ps[:, :ns],
                                         AF.Gelu,
                                         bias=ainbT[:, s * KPA + k:
                                                    s * KPA + k + 1])

            # ---- phase B: scale hidden by the gate weight (DVE) ----
            Hk = []
            for k in range(KPA):
                h = hp.tile([128, NT], BF16, tag=f"hid{k}", name=f"hid{k}")
                nc.vector.tensor_mul(h[:], Gk[k][:], wrep[:])
                Hk.append(h)

            patch_g2(0, g1s_c0)

            # ---- atom out-GEMM, grouped by dst atom ----
            # Each column range in dranges[a] holds tokens routed to atom a;
            # each range accumulates over the KPA h-shard tiles only.
            # PSUM zero regions are whole banks: the FIRST matmul into the
            # tile carries start=True (lazily zeroing the bank); every other
            # matmul accumulates — first touch of a pending byte zeroes it.
            nmm_out = sum(KPA * len(dranges[a]) for a in range(NA))
            for dp in range(DT):
                ps = pp.tile([128, 512], F32, tag="ps", name="ps")
                n = 0
                for a in range(NA):
                    for k in range(KPA):
                        for (r0, r1) in dranges[a]:
                            nc.tensor.matmul(
                                ps[:, r0:r1],
                                aoutT[a][:, k * D + dp * 128:
                                         k * D + (dp + 1) * 128],
                                Hk[k][:, r0:r1],
                                start=(n == 0), stop=(n == nmm_out - 1),
                                skip_group_check=True)
                            n += 1
                stg = op.tile([128, CW], BF16, tag="cstg", name="cstg")
                nc.vector.tensor_copy(stg[:, :NT], ps[:, :NT])
                nc.gpsimd.dma_start(cpartT_d[dp], stg[:, :NT])

            # ---- patch chunks 1..3 ----
            patch_g2(1, g1s_c1)
            xs_pre.append(load_x(2))
            patch_g2(2, patch_g1(2, xs_pre[2]))
            xs_pre.append(load_x(3))
            patch_g2(3, patch_g1(3, xs_pre[3]))

    nc.compile()
    return nc


def _sigmoid(x):
    out = np.empty_like(x)
    pos = x >= 0
    out[pos] = 1.0 / (1.0 + np.exp(-x[pos]))
    ex = np.exp(x[~pos])
    out[~pos] = ex / (1.0 + ex)
    return out


def kernel(x, patch_w1, patch_b1, patch_w2, patch_b2, gate_delta,
           atom_in_w, atom_in_b, atom_out_w, atom_out_b):
    x = np.asarray(x, dtype=np.float32)
    patch_w1 = np.asarray(patch_w1, dtype=np.float32)
    patch_b1 = np.asarray(patch_b1, dtype=np.float32)
    patch_w2 = np.asarray(patch_w2, dtype=np.float32)
    patch_b2 = np.asarray(patch_b2, dtype=np.float32)
    gate_delta = np.asarray(gate_delta, dtype=np.float32)
    atom_in_w = np.asarray(atom_in_w, dtype=np.float32)
    atom_in_b = np.asarray(atom_in_b, dtype=np.float32)
    atom_out_w = np.asarray(atom_out_w, dtype=np.float32)
    atom_out_b = np.asarray(atom_out_b, dtype=np.float32)

    bf = ml_dtypes.bfloat16

    # ---- host routing (tiny) ----
    cls3 = x[:, :NCLS, :]                                   # [B, 6, D]
    logits = np.einsum("bnd,nd->bn", cls3, gate_delta)      # [B, 6] f32
    choose_left = logits >= 0
    p_left = _sigmoid(logits)
    wgt = np.where(choose_left, p_left, 1.0 - p_left).astype(np.float32)
    keys = np.where(choose_left, LEFT_KEYS[None, :], RIGHT_KEYS[None, :])
    src = (keys // NA).reshape(-1)                          # [384]
    dst = (keys % NA).reshape(-1)
    wflat = wgt.reshape(-1)                                 # [384]

    # permute cls tokens by (src, dst) so each src atom's tokens are
    # contiguous and each dst atom's tokens are a few contiguous ranges
    order = np.lexsort((dst, src))
    inv_order = np.argsort(order)
    src_p, dst_p, wflat_p = src[order], dst[order], wflat[order]
    goff = tuple(int(np.searchsorted(src_p, s)) for s in range(NA + 1))
    dranges = []
    for a in range(NA):
        idx = np.flatnonzero(dst_p == a)
        ranges = []
        if idx.size:
            brk = np.flatnonzero(np.diff(idx) > 1)
            starts = np.concatenate(([0], brk + 1))
            ends = np.concatenate((brk, [idx.size - 1]))
            ranges = [(int(idx[s]), int(idx[e]) + 1)
                      for s, e in zip(starts, ends)]
        dranges.append(tuple(ranges))
    dranges = tuple(dranges)

    wrep_rep = np.ascontiguousarray(
        np.broadcast_to(wflat_p.reshape(1, NT), (128, NT))).astype(bf)

    # ---- replicated tensors (partition-major packed) ----
    # clsT[p, d*NT + t] = cls_permuted[t, d*128+p]
    clsT = np.ascontiguousarray(
        cls3.reshape(NT, D)[order].reshape(NT, DT, 128).transpose(2, 1, 0)
    ).reshape(128, DT * NT).astype(bf)
    # w1T[p, q, d*384 + hh*128 + m] = patch_w1[(q*3+hh)*128+m, d*128+p]
    w1T = np.ascontiguousarray(
        patch_w1.reshape(NW1P, HPP, 128, DT, 128).transpose(4, 0, 3, 1, 2)
    ).reshape(128, NW1P, DT * HPP * 128).astype(bf)
    b1T = np.ascontiguousarray(patch_b1.reshape(HT, 128).T)
    # w2T[p, h*D + dp*128 + m] = patch_w2[dp*128+m, h*128+p]
    w2T = np.ascontiguousarray(
        patch_w2.reshape(DT, 128, HT, 128).transpose(3, 2, 0, 1)
    ).reshape(128, HT * D).astype(bf)

    # ---- per-core tensors ----
    patch = x[:, NCLS:, :].reshape(NCORES, TPC, D)
    # xT[p, ci*DT*CW + d*CW + t] = patch[c][ci*CW+t, d*128+p]
    xT_all = np.ascontiguousarray(
        patch.reshape(NCORES, NCH, CW, DT, 128).transpose(0, 4, 1, 3, 2)
    ).reshape(NCORES, 128, NCH * DT * CW).astype(bf)

    ainT_all, ainbT_all, aoutT_all = [], [], []
    for c in range(NCORES):
        hsl = slice(HSH * c, HSH * (c + 1))
        # ainT[d, p, a*HSH + k*128 + m] = atom_in_w[a, hsl0 + k*128+m, d*128+p]
        ainT = np.ascontiguousarray(
            atom_in_w[:, hsl, :].reshape(NA, KPA, 128, DT, 128)
            .transpose(3, 4, 0, 1, 2)).reshape(DT, 128, NA * HSH).astype(bf)
        ainT_all.append(ainT)
        ainbT_all.append(np.ascontiguousarray(
            atom_in_b[:, hsl].reshape(HLT, 128).T))
        # aoutT[a, p, k*D + dp*128 + m] = atom_out_w[a, dp*128+m, hsl0+k*128+p]
        aoutT = np.ascontiguousarray(
            atom_out_w[:, :, hsl].reshape(NA, DT, 128, KPA, 128)
            .transpose(0, 4, 3, 1, 2)).reshape(NA, 128, KPA * D).astype(bf)
        aoutT_all.append(aoutT)

    in_maps = []
    for c in range(NCORES):
        in_maps.append({
            "xT": xT_all[c], "w1T": w1T, "b1T": b1T, "w2T": w2T,
            "clsT": clsT, "ainT": ainT_all[c], "ainbT": ainbT_all[c],
            "aoutT": aoutT_all[c], "wrep": wrep_rep,
        })

    key = (goff, dranges)
    nc = _CACHE.get(key)
    if nc is None:
        nc = _build_program(goff, dranges)
        _CACHE[key] = nc

    res = run_bass_kernel_spmd(nc, in_maps, core_ids=list(range(NCORES)))
    global LAST_RESULTS
    LAST_RESULTS = res

    # ---- host gather ----
    patch_out = np.empty((B, P, D), dtype=np.float32)
    for c in range(NCORES):
        poutT = res.results[c]["poutT"].reshape(D, TPC).astype(np.float32)
        patch_out[BPC * c:BPC * (c + 1)] = (
            poutT.T + patch_b2[None, :]).reshape(BPC, P, D)

    cpart = np.zeros((D, NT), dtype=np.float32)
    for c in range(NCORES):
        cpart += res.results[c]["cpartT"].reshape(D, NT).astype(np.float32)
    cls_out = cpart.T[inv_order] + wflat[:, None] * atom_out_b[dst, :]
    cls_out = cls_out.reshape(B, NCLS, D)

    return np.concatenate([cls_out, patch_out], axis=1)


# revision 41
# speedup vs baseline: 1.0187x; 1.0032x over previous
"""Trainium2 Bass kernel for nn_Mlp_moe: dense patch-token MLP + top-1 gated
atom (expert) routing for 6 CLS task tokens.

Sharding over 8 NeuronCores:
  - Patch MLP: data-parallel over batch B=64 -> 8 batches (1568 patch tokens)
    per core. MLP weights replicated (SBUF-resident, bf16).
  - Atom/CLS part: hidden dim H=3072 sharded 8-way (384 per core); every core
    processes all 384 CLS tokens for all 5 atoms on its H-shard and emits a
    partial output summed on the host. Routing (gate logits/sigmoid/top-1
    masks) is computed on the host (it is O(B*6*D), negligible) and shipped
    as {0,1}/weight masks folded into the device compute.

Schedule (v2): the PE is warmed with dummy matmuls from program start (HAM
clock gate releases after ~3.4us of activity), while DMAs stream the first
working set (w1 piece 0 + x chunk 0) on need-ordered queues. w1 arrives in 8
pieces so GEMM1 starts as soon as the first 3 h-tiles + x0 land. Atom phases
run between chunk0's GEMM1 and GEMM2. Outputs stage through SBUF as bf16.

Device compute is bf16 (PSUM accumulation is fp32; erf-Gelu on ScalarE is
~exact); patch outputs are bf16, cls partials fp32.
"""

import numpy as np
import ml_dtypes

import concourse.bass as bass
import concourse.bacc as bacc
import concourse.mybir as mybir
from concourse import tile
from concourse.bass_utils import run_bass_kernel_spmd

NCORES = 8
B, NCLS, P, D, H = 64, 6, 196, 768, 3072
NA = 5
HSH = H // NCORES            # 384: per-core atom hidden shard
BPC = B // NCORES            # 8 batches per core
TPC = BPC * P                # 1568 patch tokens per core
NT = B * NCLS                # 384 cls tokens
DT = D // 128                # 6 d-tiles
HT = H // 128                # 24 h-tiles
HLT = NA * HSH // 128        # 15 atom h-shard tiles (a-major, 3 per atom)
KPA = HSH // 128             # 3 h-shard tiles per atom
CW = 392
NCH = 4
CHUNKS = [(i * CW, CW) for i in range(NCH)]
NW1P = 8                     # w1 DMA pieces (3 h-tiles each)
HPP = HT // NW1P             # 3 h-tiles per piece
NWARM = 52                   # PE warm-up dummy matmuls

LEFT_KEYS = np.array([3, 4, 8, 9, 13, 14], dtype=np.int64)
RIGHT_KEYS = np.array([15, 20, 16, 21, 17, 22], dtype=np.int64)

BF16 = mybir.dt.bfloat16
F32 = mybir.dt.float32
AF = mybir.ActivationFunctionType

_CACHE = {}
LAST_RESULTS = None  # BassKernelResults of the most recent run (for profiling)


def _build_program(goff, dranges):
    """goff: 6 cumulative offsets of the 5 src-atom token groups (cls tokens
    are host-permuted by (src, dst) so each atom's tokens are a contiguous
    column range). dranges[a]: list of (start, end) column ranges whose
    tokens route their output through atom a.
    """
    nc = bacc.Bacc(None, target_bir_lowering=False, debug=False,
                   num_devices=NCORES)

    # partition-major packed inputs (see host layouts in kernel())
    xT_d = nc.dram_tensor("xT", [128, NCH * DT * CW], BF16,
                          kind="ExternalInput")
    w1T_d = nc.dram_tensor("w1T", [128, NW1P, DT * HPP * 128], BF16,
                           kind="ExternalInput")
    b1T_d = nc.dram_tensor("b1T", [128, HT], F32, kind="ExternalInput")
    w2T_d = nc.dram_tensor("w2T", [128, HT * D], BF16, kind="ExternalInput")
    clsT_d = nc.dram_tensor("clsT", [128, DT * NT], BF16,
                            kind="ExternalInput")
    ainT_d = nc.dram_tensor("ainT", [DT, 128, NA * HSH], BF16,
                            kind="ExternalInput")
    ainbT_d = nc.dram_tensor("ainbT", [128, HLT], F32, kind="ExternalInput")
    aoutT_d = nc.dram_tensor("aoutT", [NA, 128, KPA * D], BF16,
                             kind="ExternalInput")
    wrep_d = nc.dram_tensor("wrep", [128, NT], BF16, kind="ExternalInput")
    poutT_d = nc.dram_tensor("poutT", [DT, 128, TPC], BF16,
                             kind="ExternalOutput")
    cpartT_d = nc.dram_tensor("cpartT", [DT, 128, NT], BF16,
                              kind="ExternalOutput")

    with tile.TileContext(nc) as tc:
        with (
            tc.tile_pool(name="w", bufs=1) as wp,
            tc.tile_pool(name="gat", bufs=1) as gp,
            tc.tile_pool(name="hida", bufs=1) as hp,
            tc.tile_pool(name="xin", bufs=2) as xp,
            tc.tile_pool(name="g1", bufs=48) as g1p,
            tc.tile_pool(name="ostg", bufs=4) as op,
            tc.tile_pool(name="ps", bufs=7, space="PSUM") as pp,
            tc.tile_pool(name="psw", bufs=1, space="PSUM") as pwp,
        ):
            # ---- PE warm-up: dummy matmuls from program start ----
            # The HAM clock gate holds the PE at 1.2 GHz until ~3.4us of
            # sustained activity; these dummies run while the first DMAs
            # stream in so the real matmuls start at 2.4 GHz.
            wdum = wp.tile([128, 512], BF16, tag="wdum", name="wdum")
            nc.vector.memset(wdum[:], 0.03125)
            pdum = pwp.tile([128, 256], F32, tag="pdum", name="pdum")
            for _ in range(NWARM):
                nc.tensor.matmul(pdum[:], wdum[:, :128], wdum[:, :256],
                                 start=True, stop=True)

            # ---- DMA issues: global need order striped over the 3 DMA
            # queues (sync/scalar/gpsimd) so the heads of all queues are
            # always the next-needed tensors and the shared DGE engine pool
            # serves the critical path first.
            def load_x(ci):
                xa = xp.tile([128, DT * CW], BF16, tag="x", name="x")
                nc.sync.dma_start(
                    xa[:], xT_d[:, ci * DT * CW:(ci + 1) * DT * CW])
                return xa

            w1T = [wp.tile([128, DT * HPP * 128], BF16, tag=f"w1{q}",
                           name=f"w1{q}") for q in range(NW1P)]
            b1T = wp.tile([128, HT], F32, tag="b1", name="b1")
            clsT = wp.tile([128, DT * NT], BF16, tag="cls", name="cls")
            w2T = wp.tile([128, HT * D], BF16, tag="w2", name="w2")
            ainbT = wp.tile([128, HLT], F32, tag="ainb", name="ainb")
            ainT = [wp.tile([128, NA * HSH], BF16, tag=f"ain{d}",
                            name=f"ain{d}") for d in range(DT)]
            wrep = wp.tile([128, NT], BF16, tag="wr", name="wr")
            aoutT = [wp.tile([128, KPA * D], BF16, tag=f"ao{a}",
                             name=f"ao{a}") for a in range(NA)]

            # wave 1: chunk0 GEMM1 working set. x0 and every w1 piece are
            # split in d-thirds round-robined over all three queues, so the
            # shared DGE pool delivers them in exact need order at full
            # aggregate bandwidth (piece k lands before the matmuls for
            # piece k-1 complete).
            # NOTE: an engine's next dma_start blocks until its previous
            # transfer completes, so ScalarE (which must run the gelus from
            # ~16us on) gets only the 3 earliest DMAs; sync/gpsimd (no
            # compute duties) carry everything else.
            xa0 = xp.tile([128, DT * CW], BF16, tag="x", name="x")
            qs = [nc.sync, nc.scalar, nc.gpsimd]
            for i, q in enumerate(qs):
                q.dma_start(xa0[:, i * 2 * CW:(i + 1) * 2 * CW],
                            xT_d[:, i * 2 * CW:(i + 1) * 2 * CW])
            xs_pre = [xa0]
            nc.scalar.dma_start(b1T[:], b1T_d[:])
            w1c = DT * HPP * 128
            nc.scalar.dma_start(w1T[0][:, :w1c // 3], w1T_d[:, 0, :w1c // 3])
            nc.sync.dma_start(w1T[0][:, w1c // 3:2 * w1c // 3],
                              w1T_d[:, 0, w1c // 3:2 * w1c // 3])
            nc.gpsimd.dma_start(w1T[0][:, 2 * w1c // 3:],
                                w1T_d[:, 0, 2 * w1c // 3:])
            # pieces 1-7 in halves over three lanes: ScalarE's DMA lane is
            # idle from ~15us until its first gelu (~18us), so it carries two
            # early piece-halves (p1a, p3a), taking 0.6MB off the sync/gpsimd
            # streams whose delivery rate bounds when G1c0 can finish.
            lanes = {1: (nc.scalar, nc.gpsimd), 2: (nc.sync, nc.gpsimd),
                     3: (nc.scalar, nc.sync), 4: (nc.sync, nc.gpsimd),
                     5: (nc.gpsimd, nc.sync), 6: (nc.sync, nc.gpsimd),
                     7: (nc.gpsimd, nc.sync)}
            for p in range(1, NW1P):
                h1, h2 = lanes[p]
                h1.dma_start(w1T[p][:, :w1c // 2], w1T_d[:, p, :w1c // 2])
                h2.dma_start(w1T[p][:, w1c // 2:], w1T_d[:, p, w1c // 2:])
            # wave 2: chunk1 x (halves on both queues so it lands right as
            # G1c0's matmuls finish) + atom-in tensors
            xa1 = xp.tile([128, DT * CW], BF16, tag="x", name="x")
            nc.sync.dma_start(xa1[:, :3 * CW],
                              xT_d[:, DT * CW:DT * CW + 3 * CW])
            nc.gpsimd.dma_start(xa1[:, 3 * CW:],
                                xT_d[:, DT * CW + 3 * CW:2 * DT * CW])
            xs_pre.append(xa1)
            nc.gpsimd.dma_start(ainT[0][:], ainT_d[0])
            nc.sync.dma_start(clsT[:], clsT_d[:])
            nc.gpsimd.dma_start(ainT[1][:], ainT_d[1])
            nc.sync.dma_start(ainT[2][:], ainT_d[2])
            nc.gpsimd.dma_start(ainT[3][:], ainT_d[3])
            nc.sync.dma_start(ainT[4][:], ainT_d[4])
            nc.gpsimd.dma_start(ainT[5][:], ainT_d[5])
            nc.sync.dma_start(ainbT[:], ainbT_d[:])
            # wave 3: w2 + gate weights + atom-out tensors
            nc.gpsimd.dma_start(w2T[:, :12 * D], w2T_d[:, :12 * D])
            nc.sync.dma_start(w2T[:, 12 * D:], w2T_d[:, 12 * D:])
            nc.gpsimd.dma_start(wrep[:], wrep_d[:])
            nc.sync.dma_start(aoutT[0][:], aoutT_d[0])
            nc.gpsimd.dma_start(aoutT[1][:], aoutT_d[1])
            nc.sync.dma_start(aoutT[2][:], aoutT_d[2])
            nc.gpsimd.dma_start(aoutT[3][:], aoutT_d[3])
            nc.sync.dma_start(aoutT[4][:], aoutT_d[4])

            # ---- patch GEMM1 for one chunk (piece-gated on first chunk) ----
            def patch_g1(ci, xa):
                c0, cw = CHUNKS[ci]
                g1s = []
                for h in range(HT):
                    ps = pp.tile([128, 512], F32, tag="ps", name="ps")
                    q, hh = divmod(h, HPP)
                    for d in range(DT):
                        nc.tensor.matmul(
                            ps[:, :cw],
                            w1T[q][:, d * HPP * 128 + hh * 128:
                                   d * HPP * 128 + (hh + 1) * 128],
                            xa[:, d * CW:d * CW + cw],
                            start=(d == 0), stop=(d == DT - 1))
                    g1 = g1p.tile([128, CW], BF16, tag="g1", name="g1")
                    nc.scalar.activation(g1[:, :cw], ps[:, :cw], AF.Gelu,
                                         bias=b1T[:, h:h + 1])
                    g1s.append(g1)
                return g1s

            def patch_g2(ci, g1s):
                c0, cw = CHUNKS[ci]
                for dp in range(DT):
                    ps = pp.tile([128, 512], F32, tag="ps", name="ps")
                    for h in range(HT):
                        nc.tensor.matmul(
                            ps[:, :cw],
                            w2T[:, h * D + dp * 128:h * D + (dp + 1) * 128],
                            g1s[h][:, :cw],
                            start=(h == 0), stop=(h == HT - 1))
                    stg = op.tile([128, CW], BF16, tag="ostg", name="ostg")
                    nc.vector.tensor_copy(stg[:, :cw], ps[:, :cw])
                    nc.gpsimd.dma_start(poutT_d[dp][:, c0:c0 + cw],
                                        stg[:, :cw])

            g1s_c0 = patch_g1(0, xs_pre[0])
            g1s_c1 = patch_g1(1, xs_pre[1])

            # ---- phase A: grouped atom in-GEMM + gelu ----
            # cls tokens are host-permuted by src atom: group s occupies
            # columns [goff[s], goff[s+1]), so each token's hidden state is
            # computed only for its routed atom (1/5 the FLOPs of the dense
            # all-atom form).
            Gk = [gp.tile([128, NT], BF16, tag=f"g{k}", name=f"g{k}")
                  for k in range(KPA)]
            for s in range(NA):
                o0, o1 = goff[s], goff[s + 1]
                ns = o1 - o0
                if ns == 0:
                    continue
                for k in range(KPA):
                    ps = pp.tile([128, 512], F32, tag="ps", name="ps")
                    c0 = s * HSH + k * 128
                    for d in range(DT):
                        nc.tensor.matmul(
                            ps[:, :ns],
                            ainT[d][:, c0:c0 + 128],
                            clsT[:, d * NT + o0:d * NT + o1],
                            start=(d == 0), stop=(d == DT - 1))
                    nc.scalar.activation(Gk[k][:, o0:o1], ps[:, :ns],
                                         AF.Gelu,
                                         bias=ainbT[:, s * KPA + k:
                                                    s * KPA + k + 1])

            # ---- phase B: scale hidden by the gate weight (DVE) ----
            Hk = []
            for k in range(KPA):
                h = hp.tile([128, NT], BF16, tag=f"hid{k}", name=f"hid{k}")
                nc.vector.tensor_mul(h[:], Gk[k][:], wrep[:])
                Hk.append(h)

            patch_g2(0, g1s_c0)

            # ---- atom out-GEMM, grouped by dst atom ----
            # Each column range in dranges[a] holds tokens routed to atom a;
            # each range accumulates over the KPA h-shard tiles only.
            # PSUM zero regions are whole banks: the FIRST matmul into the
            # tile carries start=True (lazily zeroing the bank); every other
            # matmul accumulates — first touch of a pending byte zeroes it.
            nmm_out = sum(KPA * len(dranges[a]) for a in range(NA))
            for dp in range(DT):
                ps = pp.tile([128, 512], F32, tag="ps", name="ps")
                n = 0
                for a in range(NA):
                    for k in range(KPA):
                        for (r0, r1) in dranges[a]:
                            nc.tensor.matmul(
                                ps[:, r0:r1],
                                aoutT[a][:, k * D + dp * 128:
                                         k * D + (dp + 1) * 128],
                                Hk[k][:, r0:r1],
                                start=(n == 0), stop=(n == nmm_out - 1),
                                skip_group_check=True)
                            n += 1
                stg = op.tile([128, CW], BF16, tag="cstg", name="cstg")
                nc.vector.tensor_copy(stg[:, :NT], ps[:, :NT])
                nc.gpsimd.dma_start(cpartT_d[dp], stg[:, :NT])

            # ---- patch chunks 1..3 ----
            patch_g2(1, g1s_c1)
            xs_pre.append(load_x(2))
            patch_g2(2, patch_g1(2, xs_pre[2]))
            xs_pre.append(load_x(3))
            patch_g2(3, patch_g1(3, xs_pre[3]))

    nc.compile()
    return nc


def _sigmoid(x):
    out = np.empty_like(x)
    pos = x >= 0
    out[pos] = 1.0 / (1.0 + np.exp(-x[pos]))
    ex = np.exp(x[~pos])
    out[~pos] = ex / (1.0 + ex)
    return out


def kernel(x, patch_w1, patch_b1, patch_w2, patch_b2, gate_delta,
           atom_in_w, atom_in_b, atom_out_w, atom_out_b):
    x = np.asarray(x, dtype=np.float32)
    patch_w1 = np.asarray(patch_w1, dtype=np.float32)
    patch_b1 = np.asarray(patch_b1, dtype=np.float32)
    patch_w2 = np.asarray(patch_w2, dtype=np.float32)
    patch_b2 = np.asarray(patch_b2, dtype=np.float32)
    gate_delta = np.asarray(gate_delta, dtype=np.float32)
    atom_in_w = np.asarray(atom_in_w, dtype=np.float32)
    atom_in_b = np.asarray(atom_in_b, dtype=np.float32)
    atom_out_w = np.asarray(atom_out_w, dtype=np.float32)
    atom_out_b = np.asarray(atom_out_b, dtype=np.float32)

    bf = ml_dtypes.bfloat16

    # ---- host routing (tiny) ----
    cls3 = x[:, :NCLS, :]                                   # [B, 6, D]
    logits = np.einsum("bnd,nd->bn", cls3, gate_delta)      # [B, 6] f32
    choose_left = logits >= 0
    p_left = _sigmoid(logits)
    wgt = np.where(choose_left, p_left, 1.0 - p_left).astype(np.float32)
    keys = np.where(choose_left, LEFT_KEYS[None, :], RIGHT_KEYS[None, :])
    src = (keys // NA).reshape(-1)                          # [384]
    dst = (keys % NA).reshape(-1)
    wflat = wgt.reshape(-1)                                 # [384]

    # permute cls tokens by (src, dst) so each src atom's tokens are
    # contiguous and each dst atom's tokens are a few contiguous ranges
    order = np.lexsort((dst, src))
    inv_order = np.argsort(order)
    src_p, dst_p, wflat_p = src[order], dst[order], wflat[order]
    goff = tuple(int(np.searchsorted(src_p, s)) for s in range(NA + 1))
    dranges = []
    for a in range(NA):
        idx = np.flatnonzero(dst_p == a)
        ranges = []
        if idx.size:
            brk = np.flatnonzero(np.diff(idx) > 1)
            starts = np.concatenate(([0], brk + 1))
            ends = np.concatenate((brk, [idx.size - 1]))
            ranges = [(int(idx[s]), int(idx[e]) + 1)
                      for s, e in zip(starts, ends)]
        dranges.append(tuple(ranges))
    dranges = tuple(dranges)

    wrep_rep = np.ascontiguousarray(
        np.broadcast_to(wflat_p.reshape(1, NT), (128, NT))).astype(bf)

    # ---- replicated tensors (partition-major packed) ----
    # clsT[p, d*NT + t] = cls_permuted[t, d*128+p]
    clsT = np.ascontiguousarray(
        cls3.reshape(NT, D)[order].reshape(NT, DT, 128).transpose(2, 1, 0)
    ).reshape(128, DT * NT).astype(bf)
    # w1T[p, q, d*384 + hh*128 + m] = patch_w1[(q*3+hh)*128+m, d*128+p]
    w1T = np.ascontiguousarray(
        patch_w1.reshape(NW1P, HPP, 128, DT, 128).transpose(4, 0, 3, 1, 2)
    ).reshape(128, NW1P, DT * HPP * 128).astype(bf)
    b1T = np.ascontiguousarray(patch_b1.reshape(HT, 128).T)
    # w2T[p, h*D + dp*128 + m] = patch_w2[dp*128+m, h*128+p]
    w2T = np.ascontiguousarray(
        patch_w2.reshape(DT, 128, HT, 128).transpose(3, 2, 0, 1)
    ).reshape(128, HT * D).astype(bf)

    # ---- per-core tensors ----
    patch = x[:, NCLS:, :].reshape(NCORES, TPC, D)
    # xT[p, ci*DT*CW + d*CW + t] = patch[c][ci*CW+t, d*128+p]
    xT_all = np.ascontiguousarray(
        patch.reshape(NCORES, NCH, CW, DT, 128).transpose(0, 4, 1, 3, 2)
    ).reshape(NCORES, 128, NCH * DT * CW).astype(bf)

    ainT_all, ainbT_all, aoutT_all = [], [], []
    for c in range(NCORES):
        hsl = slice(HSH * c, HSH * (c + 1))
        # ainT[d, p, a*HSH + k*128 + m] = atom_in_w[a, hsl0 + k*128+m, d*128+p]
        ainT = np.ascontiguousarray(
            atom_in_w[:, hsl, :].reshape(NA, KPA, 128, DT, 128)
            .transpose(3, 4, 0, 1, 2)).reshape(DT, 128, NA * HSH).astype(bf)
        ainT_all.append(ainT)
        ainbT_all.append(np.ascontiguousarray(
            atom_in_b[:, hsl].reshape(HLT, 128).T))
        # aoutT[a, p, k*D + dp*128 + m] = atom_out_w[a, dp*128+m, hsl0+k*128+p]
        aoutT = np.ascontiguousarray(
            atom_out_w[:, :, hsl].reshape(NA, DT, 128, KPA, 128)
            .transpose(0, 4, 3, 1, 2)).reshape(NA, 128, KPA * D).astype(bf)
        aoutT_all.append(aoutT)

    in_maps = []
    for c in range(NCORES):
        in_maps.append({
            "xT": xT_all[c], "w1T": w1T, "b1T": b1T, "w2T": w2T,
            "clsT": clsT, "ainT": ainT_all[c], "ainbT": ainbT_all[c],
            "aoutT": aoutT_all[c], "wrep": wrep_rep,
        })

    key = (goff, dranges)
    nc = _CACHE.get(key)
    if nc is None:
        nc = _build_program(goff, dranges)
        _CACHE[key] = nc

    res = run_bass_kernel_spmd(nc, in_maps, core_ids=list(range(NCORES)))
    global LAST_RESULTS
    LAST_RESULTS = res

    # ---- host gather ----
    patch_out = np.empty((B, P, D), dtype=np.float32)
    for c in range(NCORES):
        poutT = res.results[c]["poutT"].reshape(D, TPC).astype(np.float32)
        patch_out[BPC * c:BPC * (c + 1)] = (
            poutT.T + patch_b2[None, :]).reshape(BPC, P, D)

    cpart = np.zeros((D, NT), dtype=np.float32)
    for c in range(NCORES):
        cpart += res.results[c]["cpartT"].reshape(D, NT).astype(np.float32)
    cls_out = cpart.T[inv_order] + wflat[:, None] * atom_out_b[dst, :]
    cls_out = cls_out.reshape(B, NCLS, D)

    return np.concatenate([cls_out, patch_out], axis=1)


# revision 45
# speedup vs baseline: 1.0290x; 1.0101x over previous
"""Trainium2 Bass kernel for nn_Mlp_moe: dense patch-token MLP + top-1 gated
atom (expert) routing for 6 CLS task tokens.

Sharding over 8 NeuronCores:
  - Patch MLP: data-parallel over batch B=64 -> 8 batches (1568 patch tokens)
    per core. MLP weights replicated (SBUF-resident, bf16).
  - Atom/CLS part: hidden dim H=3072 sharded 8-way (384 per core); every core
    processes all 384 CLS tokens for all 5 atoms on its H-shard and emits a
    partial output summed on the host. Routing (gate logits/sigmoid/top-1
    masks) is computed on the host (it is O(B*6*D), negligible) and shipped
    as {0,1}/weight masks folded into the device compute.

Schedule (v2): the PE is warmed with dummy matmuls from program start (HAM
clock gate releases after ~3.4us of activity), while DMAs stream the first
working set (w1 piece 0 + x chunk 0) on need-ordered queues. w1 arrives in 8
pieces so GEMM1 starts as soon as the first 3 h-tiles + x0 land. Atom phases
run between chunk0's GEMM1 and GEMM2. Outputs stage through SBUF as bf16.

Device compute is bf16 (PSUM accumulation is fp32; erf-Gelu on ScalarE is
~exact); patch outputs are bf16, cls partials fp32.
"""

import numpy as np
import ml_dtypes

import concourse.bass as bass
import concourse.bacc as bacc
import concourse.mybir as mybir
from concourse import tile
from concourse.bass_utils import run_bass_kernel_spmd

NCORES = 8
B, NCLS, P, D, H = 64, 6, 196, 768, 3072
NA = 5
HSH = H // NCORES            # 384: per-core atom hidden shard
BPC = B // NCORES            # 8 batches per core
TPC = BPC * P                # 1568 patch tokens per core
NT = B * NCLS                # 384 cls tokens
DT = D // 128                # 6 d-tiles
HT = H // 128                # 24 h-tiles
HLT = NA * HSH // 128        # 15 atom h-shard tiles (a-major, 3 per atom)
KPA = HSH // 128             # 3 h-shard tiles per atom
CW = 392
NCH = 4
CHUNKS = [(i * CW, CW) for i in range(NCH)]
NW1P = 8                     # w1 DMA pieces (3 h-tiles each)
HPP = HT // NW1P             # 3 h-tiles per piece
NWARM = 30                   # PE warm-up dummy matmuls

LEFT_KEYS = np.array([3, 4, 8, 9, 13, 14], dtype=np.int64)
RIGHT_KEYS = np.array([15, 20, 16, 21, 17, 22], dtype=np.int64)

BF16 = mybir.dt.bfloat16
F32 = mybir.dt.float32
AF = mybir.ActivationFunctionType

_CACHE = {}
LAST_RESULTS = None  # BassKernelResults of the most recent run (for profiling)


def _build_program(goff, dranges):
    """goff: 6 cumulative offsets of the 5 src-atom token groups (cls tokens
    are host-permuted by (src, dst) so each atom's tokens are a contiguous
    column range). dranges[a]: list of (start, end) column ranges whose
    tokens route their output through atom a.
    """
    nc = bacc.Bacc(None, target_bir_lowering=False, debug=False,
                   num_devices=NCORES)

    # partition-major packed inputs (see host layouts in kernel())
    xT_d = nc.dram_tensor("xT", [128, NCH * DT * CW], BF16,
                          kind="ExternalInput")
    w1T_d = nc.dram_tensor("w1T", [128, NW1P, DT * HPP * 128], BF16,
                           kind="ExternalInput")
    b1T_d = nc.dram_tensor("b1T", [128, HT], F32, kind="ExternalInput")
    w2T_d = nc.dram_tensor("w2T", [128, HT * D], BF16, kind="ExternalInput")
    clsT_d = nc.dram_tensor("clsT", [128, DT * NT], BF16,
                            kind="ExternalInput")
    ainT_d = nc.dram_tensor("ainT", [DT, 128, NA * HSH], BF16,
                            kind="ExternalInput")
    ainbT_d = nc.dram_tensor("ainbT", [128, HLT], F32, kind="ExternalInput")
    aoutT_d = nc.dram_tensor("aoutT", [NA, 128, KPA * D], BF16,
                             kind="ExternalInput")
    wrep_d = nc.dram_tensor("wrep", [128, NT], BF16, kind="ExternalInput")
    poutT_d = nc.dram_tensor("poutT", [DT, 128, TPC], BF16,
                             kind="ExternalOutput")
    cpartT_d = nc.dram_tensor("cpartT", [DT, 128, NT], BF16,
                              kind="ExternalOutput")

    with tile.TileContext(nc) as tc:
        with (
            tc.tile_pool(name="w", bufs=1) as wp,
            tc.tile_pool(name="gat", bufs=1) as gp,
            tc.tile_pool(name="hida", bufs=1) as hp,
            tc.tile_pool(name="xin", bufs=2) as xp,
            tc.tile_pool(name="g1", bufs=48) as g1p,
            tc.tile_pool(name="ostg", bufs=4) as op,
            tc.tile_pool(name="ps", bufs=7, space="PSUM") as pp,
            tc.tile_pool(name="psw", bufs=1, space="PSUM") as pwp,
        ):
            # ---- PE warm-up: dummy matmuls from program start ----
            # The HAM clock gate holds the PE at 1.2 GHz until ~3.4us of
            # sustained activity; these dummies run while the first DMAs
            # stream in so the real matmuls start at 2.4 GHz.
            wdum = wp.tile([128, 512], BF16, tag="wdum", name="wdum")
            nc.vector.memset(wdum[:], 0.03125)
            pdum = pwp.tile([128, 256], F32, tag="pdum", name="pdum")
            for _ in range(NWARM):
                nc.tensor.matmul(pdum[:], wdum[:, :128], wdum[:, :256],
                                 start=True, stop=True)

            # ---- DMA issues: global need order striped over the 3 DMA
            # queues (sync/scalar/gpsimd) so the heads of all queues are
            # always the next-needed tensors and the shared DGE engine pool
            # serves the critical path first.
            def load_x(ci):
                xa = xp.tile([128, DT * CW], BF16, tag="x", name="x")
                nc.sync.dma_start(
                    xa[:], xT_d[:, ci * DT * CW:(ci + 1) * DT * CW])
                return xa

            w1T = [wp.tile([128, DT * HPP * 128], BF16, tag=f"w1{q}",
                           name=f"w1{q}") for q in range(NW1P)]
            b1T = wp.tile([128, HT], F32, tag="b1", name="b1")
            clsT = wp.tile([128, DT * NT], BF16, tag="cls", name="cls")
            w2T = wp.tile([128, HT * D], BF16, tag="w2", name="w2")
            ainbT = wp.tile([128, HLT], F32, tag="ainb", name="ainb")
            ainT = [wp.tile([128, NA * HSH], BF16, tag=f"ain{d}",
                            name=f"ain{d}") for d in range(DT)]
            wrep = wp.tile([128, NT], BF16, tag="wr", name="wr")
            aoutT = [wp.tile([128, KPA * D], BF16, tag=f"ao{a}",
                             name=f"ao{a}") for a in range(NA)]

            # wave 1: chunk0 GEMM1 working set. x0 and every w1 piece are
            # split in d-thirds round-robined over all three queues, so the
            # shared DGE pool delivers them in exact need order at full
            # aggregate bandwidth (piece k lands before the matmuls for
            # piece k-1 complete).
            # NOTE: an engine's next dma_start blocks until its previous
            # transfer completes, so ScalarE (which must run the gelus from
            # ~16us on) gets only the 3 earliest DMAs; sync/gpsimd (no
            # compute duties) carry everything else.
            # first-matmul critical set: w1 piece0's hh0 third rides ScalarE
            # first, x0 halves ride sync/gpsimd firsts -> first MM ~13us
            xa0 = xp.tile([128, DT * CW], BF16, tag="x", name="x")
            w1c = DT * HPP * 128
            nc.scalar.dma_start(w1T[0][:, :w1c // 3], w1T_d[:, 0, :w1c // 3])
            nc.sync.dma_start(xa0[:, :3 * CW], xT_d[:, :3 * CW])
            nc.gpsimd.dma_start(xa0[:, 3 * CW:DT * CW],
                                xT_d[:, 3 * CW:DT * CW])
            xs_pre = [xa0]
            nc.scalar.dma_start(b1T[:], b1T_d[:])
            nc.sync.dma_start(w1T[0][:, w1c // 3:2 * w1c // 3],
                              w1T_d[:, 0, w1c // 3:2 * w1c // 3])
            nc.gpsimd.dma_start(w1T[0][:, 2 * w1c // 3:],
                                w1T_d[:, 0, 2 * w1c // 3:])
            # pieces 1-7 in halves over three lanes: ScalarE's DMA lane is
            # idle from ~15us until its first gelu (~18us), so it carries two
            # early piece-halves (p1a, p3a), taking 0.6MB off the sync/gpsimd
            # streams whose delivery rate bounds when G1c0 can finish.
            lanes = {1: (nc.scalar, nc.gpsimd), 2: (nc.sync, nc.gpsimd),
                     3: (nc.scalar, nc.sync), 4: (nc.sync, nc.gpsimd),
                     5: (nc.gpsimd, nc.sync), 6: (nc.sync, nc.gpsimd),
                     7: (nc.gpsimd, nc.sync)}
            for p in range(1, NW1P):
                h1, h2 = lanes[p]
                h1.dma_start(w1T[p][:, :w1c // 2], w1T_d[:, p, :w1c // 2])
                h2.dma_start(w1T[p][:, w1c // 2:], w1T_d[:, p, w1c // 2:])
            # wave 2: chunk1 x (halves on both queues so it lands right as
            # G1c0's matmuls finish) + atom-in tensors
            xa1 = xp.tile([128, DT * CW], BF16, tag="x", name="x")
            nc.sync.dma_start(xa1[:, :3 * CW],
                              xT_d[:, DT * CW:DT * CW + 3 * CW])
            nc.gpsimd.dma_start(xa1[:, 3 * CW:],
                                xT_d[:, DT * CW + 3 * CW:2 * DT * CW])
            xs_pre.append(xa1)
            nc.gpsimd.dma_start(ainT[0][:], ainT_d[0])
            nc.sync.dma_start(clsT[:], clsT_d[:])
            nc.gpsimd.dma_start(ainT[1][:], ainT_d[1])
            nc.sync.dma_start(ainT[2][:], ainT_d[2])
            nc.gpsimd.dma_start(ainT[3][:], ainT_d[3])
            nc.sync.dma_start(ainT[4][:], ainT_d[4])
            nc.gpsimd.dma_start(ainT[5][:], ainT_d[5])
            nc.sync.dma_start(ainbT[:], ainbT_d[:])
            # wave 3: w2 + gate weights + atom-out tensors
            nc.gpsimd.dma_start(w2T[:, :12 * D], w2T_d[:, :12 * D])
            nc.sync.dma_start(w2T[:, 12 * D:], w2T_d[:, 12 * D:])
            nc.gpsimd.dma_start(wrep[:], wrep_d[:])
            nc.sync.dma_start(aoutT[0][:], aoutT_d[0])
            nc.gpsimd.dma_start(aoutT[1][:], aoutT_d[1])
            nc.sync.dma_start(aoutT[2][:], aoutT_d[2])
            nc.gpsimd.dma_start(aoutT[3][:], aoutT_d[3])
            nc.sync.dma_start(aoutT[4][:], aoutT_d[4])

            # ---- patch GEMM1 for one chunk (piece-gated on first chunk) ----
            def patch_g1(ci, xa):
                c0, cw = CHUNKS[ci]
                g1s = []
                for h in range(HT):
                    ps = pp.tile([128, 512], F32, tag="ps", name="ps")
                    q, hh = divmod(h, HPP)
                    for d in range(DT):
                        nc.tensor.matmul(
                            ps[:, :cw],
                            w1T[q][:, hh * DT * 128 + d * 128:
                                   hh * DT * 128 + (d + 1) * 128],
                            xa[:, d * CW:d * CW + cw],
                            start=(d == 0), stop=(d == DT - 1))
                    g1 = g1p.tile([128, CW], BF16, tag="g1", name="g1")
                    nc.scalar.activation(g1[:, :cw], ps[:, :cw], AF.Gelu,
                                         bias=b1T[:, h:h + 1])
                    g1s.append(g1)
                return g1s

            def patch_g2(ci, g1s):
                c0, cw = CHUNKS[ci]
                for dp in range(DT):
                    ps = pp.tile([128, 512], F32, tag="ps", name="ps")
                    for h in range(HT):
                        nc.tensor.matmul(
                            ps[:, :cw],
                            w2T[:, h * D + dp * 128:h * D + (dp + 1) * 128],
                            g1s[h][:, :cw],
                            start=(h == 0), stop=(h == HT - 1))
                    stg = op.tile([128, CW], BF16, tag="ostg", name="ostg")
                    nc.vector.tensor_copy(stg[:, :cw], ps[:, :cw])
                    nc.gpsimd.dma_start(poutT_d[dp][:, c0:c0 + cw],
                                        stg[:, :cw])

            g1s_c0 = patch_g1(0, xs_pre[0])
            g1s_c1 = patch_g1(1, xs_pre[1])

            # ---- phase A: grouped atom in-GEMM + gelu ----
            # cls tokens are host-permuted by src atom: group s occupies
            # columns [goff[s], goff[s+1]), so each token's hidden state is
            # computed only for its routed atom (1/5 the FLOPs of the dense
            # all-atom form).
            Gk = [gp.tile([128, NT], BF16, tag=f"g{k}", name=f"g{k}")
                  for k in range(KPA)]
            for s in range(NA):
                o0, o1 = goff[s], goff[s + 1]
                ns = o1 - o0
                if ns == 0:
                    continue
                for k in range(KPA):
                    ps = pp.tile([128, 512], F32, tag="ps", name="ps")
                    c0 = s * HSH + k * 128
                    for d in range(DT):
                        nc.tensor.matmul(
                            ps[:, :ns],
                            ainT[d][:, c0:c0 + 128],
                            clsT[:, d * NT + o0:d * NT + o1],
                            start=(d == 0), stop=(d == DT - 1))
                    nc.scalar.activation(Gk[k][:, o0:o1], ps[:, :ns],
                                         AF.Gelu,
                                         bias=ainbT[:, s * KPA + k:
                                                    s * KPA + k + 1])

            # ---- phase B: scale hidden by the gate weight (DVE) ----
            Hk = []
            for k in range(KPA):
                h = hp.tile([128, NT], BF16, tag=f"hid{k}", name=f"hid{k}")
                nc.vector.tensor_mul(h[:], Gk[k][:], wrep[:])
                Hk.append(h)

            patch_g2(0, g1s_c0)

            # ---- atom out-GEMM, grouped by dst atom ----
            # Each column range in dranges[a] holds tokens routed to atom a;
            # each range accumulates over the KPA h-shard tiles only.
            # PSUM zero regions are whole banks: the FIRST matmul into the
            # tile carries start=True (lazily zeroing the bank); every other
            # matmul accumulates — first touch of a pending byte zeroes it.
            nmm_out = sum(KPA * len(dranges[a]) for a in range(NA))
            for dp in range(DT):
                ps = pp.tile([128, 512], F32, tag="ps", name="ps")
                n = 0
                for a in range(NA):
                    for k in range(KPA):
                        for (r0, r1) in dranges[a]:
                            nc.tensor.matmul(
                                ps[:, r0:r1],
                                aoutT[a][:, k * D + dp * 128:
                                         k * D + (dp + 1) * 128],
                                Hk[k][:, r0:r1],
                                start=(n == 0), stop=(n == nmm_out - 1),
                                skip_group_check=True)
                            n += 1
                stg = op.tile([128, CW], BF16, tag="cstg", name="cstg")
                nc.vector.tensor_copy(stg[:, :NT], ps[:, :NT])
                nc.gpsimd.dma_start(cpartT_d[dp], stg[:, :NT])

            # ---- patch chunks 1..3 ----
            patch_g2(1, g1s_c1)
            xs_pre.append(load_x(2))
            patch_g2(2, patch_g1(2, xs_pre[2]))
            xs_pre.append(load_x(3))
            patch_g2(3, patch_g1(3, xs_pre[3]))

    nc.compile()
    return nc


def _sigmoid(x):
    out = np.empty_like(x)
    pos = x >= 0
    out[pos] = 1.0 / (1.0 + np.exp(-x[pos]))
    ex = np.exp(x[~pos])
    out[~pos] = ex / (1.0 + ex)
    return out


def kernel(x, patch_w1, patch_b1, patch_w2, patch_b2, gate_delta,
           atom_in_w, atom_in_b, atom_out_w, atom_out_b):
    x = np.asarray(x, dtype=np.float32)
    patch_w1 = np.asarray(patch_w1, dtype=np.float32)
    patch_b1 = np.asarray(patch_b1, dtype=np.float32)
    patch_w2 = np.asarray(patch_w2, dtype=np.float32)
    patch_b2 = np.asarray(patch_b2, dtype=np.float32)
    gate_delta = np.asarray(gate_delta, dtype=np.float32)
    atom_in_w = np.asarray(atom_in_w, dtype=np.float32)
    atom_in_b = np.asarray(atom_in_b, dtype=np.float32)
    atom_out_w = np.asarray(atom_out_w, dtype=np.float32)
    atom_out_b = np.asarray(atom_out_b, dtype=np.float32)

    bf = ml_dtypes.bfloat16

    # ---- host routing (tiny) ----
    cls3 = x[:, :NCLS, :]                                   # [B, 6, D]
    logits = np.einsum("bnd,nd->bn", cls3, gate_delta)      # [B, 6] f32
    choose_left = logits >= 0
    p_left = _sigmoid(logits)
    wgt = np.where(choose_left, p_left, 1.0 - p_left).astype(np.float32)
    keys = np.where(choose_left, LEFT_KEYS[None, :], RIGHT_KEYS[None, :])
    src = (keys // NA).reshape(-1)                          # [384]
    dst = (keys % NA).reshape(-1)
    wflat = wgt.reshape(-1)                                 # [384]

    # permute cls tokens by (src, dst) so each src atom's tokens are
    # contiguous and each dst atom's tokens are a few contiguous ranges
    order = np.lexsort((dst, src))
    inv_order = np.argsort(order)
    src_p, dst_p, wflat_p = src[order], dst[order], wflat[order]
    goff = tuple(int(np.searchsorted(src_p, s)) for s in range(NA + 1))
    dranges = []
    for a in range(NA):
        idx = np.flatnonzero(dst_p == a)
        ranges = []
        if idx.size:
            brk = np.flatnonzero(np.diff(idx) > 1)
            starts = np.concatenate(([0], brk + 1))
            ends = np.concatenate((brk, [idx.size - 1]))
            ranges = [(int(idx[s]), int(idx[e]) + 1)
                      for s, e in zip(starts, ends)]
        dranges.append(tuple(ranges))
    dranges = tuple(dranges)

    wrep_rep = np.ascontiguousarray(
        np.broadcast_to(wflat_p.reshape(1, NT), (128, NT))).astype(bf)

    # ---- replicated tensors (partition-major packed) ----
    # clsT[p, d*NT + t] = cls_permuted[t, d*128+p]
    clsT = np.ascontiguousarray(
        cls3.reshape(NT, D)[order].reshape(NT, DT, 128).transpose(2, 1, 0)
    ).reshape(128, DT * NT).astype(bf)
    # w1T[p, q, hh*768 + d*128 + m] = patch_w1[(q*3+hh)*128+m, d*128+p]
    # (hh-major within each piece so the first-needed h-tile is the piece's
    # first contiguous third)
    w1T = np.ascontiguousarray(
        patch_w1.reshape(NW1P, HPP, 128, DT, 128).transpose(4, 0, 1, 3, 2)
    ).reshape(128, NW1P, DT * HPP * 128).astype(bf)
    b1T = np.ascontiguousarray(patch_b1.reshape(HT, 128).T)
    # w2T[p, h*D + dp*128 + m] = patch_w2[dp*128+m, h*128+p]
    w2T = np.ascontiguousarray(
        patch_w2.reshape(DT, 128, HT, 128).transpose(3, 2, 0, 1)
    ).reshape(128, HT * D).astype(bf)

    # ---- per-core tensors ----
    patch = x[:, NCLS:, :].reshape(NCORES, TPC, D)
    # xT[p, ci*DT*CW + d*CW + t] = patch[c][ci*CW+t, d*128+p]
    xT_all = np.ascontiguousarray(
        patch.reshape(NCORES, NCH, CW, DT, 128).transpose(0, 4, 1, 3, 2)
    ).reshape(NCORES, 128, NCH * DT * CW).astype(bf)

    ainT_all, ainbT_all, aoutT_all = [], [], []
    for c in range(NCORES):
        hsl = slice(HSH * c, HSH * (c + 1))
        # ainT[d, p, a*HSH + k*128 + m] = atom_in_w[a, hsl0 + k*128+m, d*128+p]
        ainT = np.ascontiguousarray(
            atom_in_w[:, hsl, :].reshape(NA, KPA, 128, DT, 128)
            .transpose(3, 4, 0, 1, 2)).reshape(DT, 128, NA * HSH).astype(bf)
        ainT_all.append(ainT)
        ainbT_all.append(np.ascontiguousarray(
            atom_in_b[:, hsl].reshape(HLT, 128).T))
        # aoutT[a, p, k*D + dp*128 + m] = atom_out_w[a, dp*128+m, hsl0+k*128+p]
        aoutT = np.ascontiguousarray(
            atom_out_w[:, :, hsl].reshape(NA, DT, 128, KPA, 128)
            .transpose(0, 4, 3, 1, 2)).reshape(NA, 128, KPA * D).astype(bf)
        aoutT_all.append(aoutT)

    in_maps = []
    for c in range(NCORES):
        in_maps.append({
            "xT": xT_all[c], "w1T": w1T, "b1T": b1T, "w2T": w2T,
            "clsT": clsT, "ainT": ainT_all[c], "ainbT": ainbT_all[c],
            "aoutT": aoutT_all[c], "wrep": wrep_rep,
        })

    key = (goff, dranges)
    nc = _CACHE.get(key)
    if nc is None:
        nc = _build_program(goff, dranges)
        _CACHE[key] = nc

    res = run_bass_kernel_spmd(nc, in_maps, core_ids=list(range(NCORES)))
    global LAST_RESULTS
    LAST_RESULTS = res

    # ---- host gather ----
    patch_out = np.empty((B, P, D), dtype=np.float32)
    for c in range(NCORES):
        poutT = res.results[c]["poutT"].reshape(D, TPC).astype(np.float32)
        patch_out[BPC * c:BPC * (c + 1)] = (
            poutT.T + patch_b2[None, :]).reshape(BPC, P, D)

    cpart = np.zeros((D, NT), dtype=np.float32)
    for c in range(NCORES):
        cpart += res.results[c]["cpartT"].reshape(D, NT).astype(np.float32)
    cls_out = cpart.T[inv_order] + wflat[:, None] * atom_out_b[dst, :]
    cls_out = cls_out.reshape(B, NCLS, D)

    return np.concatenate([cls_out, patch_out], axis=1)


# revision 47
# speedup vs baseline: 1.0295x; 1.0005x over previous
"""Trainium2 Bass kernel for nn_Mlp_moe: dense patch-token MLP + top-1 gated
atom (expert) routing for 6 CLS task tokens.

Sharding over 8 NeuronCores:
  - Patch MLP: data-parallel over batch B=64 -> 8 batches (1568 patch tokens)
    per core. MLP weights replicated (SBUF-resident, bf16).
  - Atom/CLS part: hidden dim H=3072 sharded 8-way (384 per core); every core
    processes all 384 CLS tokens for all 5 atoms on its H-shard and emits a
    partial output summed on the host. Routing (gate logits/sigmoid/top-1
    masks) is computed on the host (it is O(B*6*D), negligible) and shipped
    as {0,1}/weight masks folded into the device compute.

Schedule (v2): the PE is warmed with dummy matmuls from program start (HAM
clock gate releases after ~3.4us of activity), while DMAs stream the first
working set (w1 piece 0 + x chunk 0) on need-ordered queues. w1 arrives in 8
pieces so GEMM1 starts as soon as the first 3 h-tiles + x0 land. Atom phases
run between chunk0's GEMM1 and GEMM2. Outputs stage through SBUF as bf16.

Device compute is bf16 (PSUM accumulation is fp32; erf-Gelu on ScalarE is
~exact); patch outputs are bf16, cls partials fp32.
"""

import numpy as np
import ml_dtypes

import concourse.bass as bass
import concourse.bacc as bacc
import concourse.mybir as mybir
from concourse import tile
from concourse.bass_utils import run_bass_kernel_spmd

NCORES = 8
B, NCLS, P, D, H = 64, 6, 196, 768, 3072
NA = 5
HSH = H // NCORES            # 384: per-core atom hidden shard
BPC = B // NCORES            # 8 batches per core
TPC = BPC * P                # 1568 patch tokens per core
NT = B * NCLS                # 384 cls tokens
DT = D // 128                # 6 d-tiles
HT = H // 128                # 24 h-tiles
HLT = NA * HSH // 128        # 15 atom h-shard tiles (a-major, 3 per atom)
KPA = HSH // 128             # 3 h-shard tiles per atom
CW = 392
NCH = 4
CHUNKS = [(i * CW, CW) for i in range(NCH)]
NW1P = 8                     # w1 DMA pieces (3 h-tiles each)
HPP = HT // NW1P             # 3 h-tiles per piece
NWARM = 30                   # PE warm-up dummy matmuls

LEFT_KEYS = np.array([3, 4, 8, 9, 13, 14], dtype=np.int64)
RIGHT_KEYS = np.array([15, 20, 16, 21, 17, 22], dtype=np.int64)

BF16 = mybir.dt.bfloat16
F32 = mybir.dt.float32
AF = mybir.ActivationFunctionType

_CACHE = {}
LAST_RESULTS = None  # BassKernelResults of the most recent run (for profiling)


def _build_program(goff, dranges):
    """goff: 6 cumulative offsets of the 5 src-atom token groups (cls tokens
    are host-permuted by (src, dst) so each atom's tokens are a contiguous
    column range). dranges[a]: list of (start, end) column ranges whose
    tokens route their output through atom a.
    """
    nc = bacc.Bacc(None, target_bir_lowering=False, debug=False,
                   num_devices=NCORES)

    # partition-major packed inputs (see host layouts in kernel())
    xT_d = nc.dram_tensor("xT", [128, NCH * DT * CW], BF16,
                          kind="ExternalInput")
    w1T_d = nc.dram_tensor("w1T", [128, NW1P, DT * HPP * 128], BF16,
                           kind="ExternalInput")
    b1T_d = nc.dram_tensor("b1T", [128, HT], F32, kind="ExternalInput")
    w2T_d = nc.dram_tensor("w2T", [128, HT * D], BF16, kind="ExternalInput")
    clsT_d = nc.dram_tensor("clsT", [128, DT * NT], BF16,
                            kind="ExternalInput")
    ainT_d = nc.dram_tensor("ainT", [DT, 128, NA * HSH], BF16,
                            kind="ExternalInput")
    ainbT_d = nc.dram_tensor("ainbT", [128, HLT], F32, kind="ExternalInput")
    aoutT_d = nc.dram_tensor("aoutT", [NA, 128, KPA * D], BF16,
                             kind="ExternalInput")
    wrep_d = nc.dram_tensor("wrep", [128, NT], BF16, kind="ExternalInput")
    poutT_d = nc.dram_tensor("poutT", [DT, 128, TPC], BF16,
                             kind="ExternalOutput")
    cpartT_d = nc.dram_tensor("cpartT", [DT, 128, NT], BF16,
                              kind="ExternalOutput")

    with tile.TileContext(nc) as tc:
        with (
            tc.tile_pool(name="w", bufs=1) as wp,
            tc.tile_pool(name="gat", bufs=1) as gp,
            tc.tile_pool(name="hida", bufs=1) as hp,
            tc.tile_pool(name="xin", bufs=2) as xp,
            tc.tile_pool(name="g1", bufs=48) as g1p,
            tc.tile_pool(name="ostg", bufs=4) as op,
            tc.tile_pool(name="ps", bufs=7, space="PSUM") as pp,
            tc.tile_pool(name="psw", bufs=1, space="PSUM") as pwp,
        ):
            # ---- PE warm-up: dummy matmuls from program start ----
            # The HAM clock gate holds the PE at 1.2 GHz until ~3.4us of
            # sustained activity; these dummies run while the first DMAs
            # stream in so the real matmuls start at 2.4 GHz.
            wdum = wp.tile([128, 512], BF16, tag="wdum", name="wdum")
            nc.vector.memset(wdum[:], 0.03125)
            pdum = pwp.tile([128, 256], F32, tag="pdum", name="pdum")
            for _ in range(NWARM):
                nc.tensor.matmul(pdum[:], wdum[:, :128], wdum[:, :256],
                                 start=True, stop=True)

            # ---- DMA issues: global need order striped over the 3 DMA
            # queues (sync/scalar/gpsimd) so the heads of all queues are
            # always the next-needed tensors and the shared DGE engine pool
            # serves the critical path first.
            def load_x(ci):
                xa = xp.tile([128, DT * CW], BF16, tag="x", name="x")
                nc.sync.dma_start(
                    xa[:], xT_d[:, ci * DT * CW:(ci + 1) * DT * CW])
                return xa

            w1T = [wp.tile([128, DT * HPP * 128], BF16, tag=f"w1{q}",
                           name=f"w1{q}") for q in range(NW1P)]
            b1T = wp.tile([128, HT], F32, tag="b1", name="b1")
            clsT = wp.tile([128, DT * NT], BF16, tag="cls", name="cls")
            w2T = wp.tile([128, HT * D], BF16, tag="w2", name="w2")
            ainbT = wp.tile([128, HLT], F32, tag="ainb", name="ainb")
            ainT = [wp.tile([128, NA * HSH], BF16, tag=f"ain{d}",
                            name=f"ain{d}") for d in range(DT)]
            wrep = wp.tile([128, NT], BF16, tag="wr", name="wr")
            aoutT = [wp.tile([128, KPA * D], BF16, tag=f"ao{a}",
                             name=f"ao{a}") for a in range(NA)]

            # wave 1: chunk0 GEMM1 working set. x0 and every w1 piece are
            # split in d-thirds round-robined over all three queues, so the
            # shared DGE pool delivers them in exact need order at full
            # aggregate bandwidth (piece k lands before the matmuls for
            # piece k-1 complete).
            # NOTE: an engine's next dma_start blocks until its previous
            # transfer completes, so ScalarE (which must run the gelus from
            # ~16us on) gets only the 3 earliest DMAs; sync/gpsimd (no
            # compute duties) carry everything else.
            # first-matmul critical set: w1 piece0's hh0 third rides ScalarE
            # first, x0 halves ride sync/gpsimd firsts -> first MM ~13us
            xa0 = xp.tile([128, DT * CW], BF16, tag="x", name="x")
            w1c = DT * HPP * 128
            nc.scalar.dma_start(w1T[0][:, :w1c // 3], w1T_d[:, 0, :w1c // 3])
            nc.sync.dma_start(xa0[:, :3 * CW], xT_d[:, :3 * CW])
            nc.gpsimd.dma_start(xa0[:, 3 * CW:DT * CW],
                                xT_d[:, 3 * CW:DT * CW])
            xs_pre = [xa0]
            nc.scalar.dma_start(b1T[:], b1T_d[:])
            nc.sync.dma_start(w1T[0][:, w1c // 3:2 * w1c // 3],
                              w1T_d[:, 0, w1c // 3:2 * w1c // 3])
            nc.gpsimd.dma_start(w1T[0][:, 2 * w1c // 3:],
                                w1T_d[:, 0, 2 * w1c // 3:])
            # pieces 1-7 in halves over three lanes: ScalarE's DMA lane is
            # idle from ~15us until its first gelu (~18us), so it carries two
            # early piece-halves (p1a, p3a), taking 0.6MB off the sync/gpsimd
            # streams whose delivery rate bounds when G1c0 can finish.
            lanes = {1: (nc.scalar, nc.gpsimd), 2: (nc.sync, nc.gpsimd),
                     3: (nc.scalar, nc.sync), 4: (nc.sync, nc.gpsimd),
                     5: (nc.gpsimd, nc.sync), 6: (nc.sync, nc.gpsimd),
                     7: (nc.gpsimd, nc.sync)}
            for p in range(1, NW1P):
                h1, h2 = lanes[p]
                h1.dma_start(w1T[p][:, :w1c // 2], w1T_d[:, p, :w1c // 2])
                h2.dma_start(w1T[p][:, w1c // 2:], w1T_d[:, p, w1c // 2:])
            # wave 2: chunk1 x (halves on both queues so it lands right as
            # G1c0's matmuls finish) + atom-in tensors
            xa1 = xp.tile([128, DT * CW], BF16, tag="x", name="x")
            nc.sync.dma_start(xa1[:, :3 * CW],
                              xT_d[:, DT * CW:DT * CW + 3 * CW])
            nc.gpsimd.dma_start(xa1[:, 3 * CW:],
                                xT_d[:, DT * CW + 3 * CW:2 * DT * CW])
            xs_pre.append(xa1)
            nc.gpsimd.dma_start(ainT[0][:], ainT_d[0])
            nc.sync.dma_start(clsT[:], clsT_d[:])
            nc.gpsimd.dma_start(ainT[1][:], ainT_d[1])
            nc.sync.dma_start(ainT[2][:], ainT_d[2])
            nc.gpsimd.dma_start(ainT[3][:], ainT_d[3])
            nc.sync.dma_start(ainT[4][:], ainT_d[4])
            nc.gpsimd.dma_start(ainT[5][:], ainT_d[5])
            nc.sync.dma_start(ainbT[:], ainbT_d[:])
            # wave 3: w2 + gate weights + atom-out tensors
            nc.gpsimd.dma_start(w2T[:, :12 * D], w2T_d[:, :12 * D])
            nc.sync.dma_start(w2T[:, 12 * D:], w2T_d[:, 12 * D:])
            nc.gpsimd.dma_start(wrep[:], wrep_d[:])
            nc.sync.dma_start(aoutT[0][:], aoutT_d[0])
            nc.gpsimd.dma_start(aoutT[1][:], aoutT_d[1])
            nc.sync.dma_start(aoutT[2][:], aoutT_d[2])
            nc.gpsimd.dma_start(aoutT[3][:], aoutT_d[3])
            nc.sync.dma_start(aoutT[4][:], aoutT_d[4])

            # ---- patch GEMM1 for one chunk (piece-gated on first chunk) ----
            def patch_g1(ci, xa):
                c0, cw = CHUNKS[ci]
                g1s = []
                for h in range(HT):
                    ps = pp.tile([128, 512], F32, tag="ps", name="ps")
                    q, hh = divmod(h, HPP)
                    for d in range(DT):
                        nc.tensor.matmul(
                            ps[:, :cw],
                            w1T[q][:, hh * DT * 128 + d * 128:
                                   hh * DT * 128 + (d + 1) * 128],
                            xa[:, d * CW:d * CW + cw],
                            start=(d == 0), stop=(d == DT - 1))
                    g1 = g1p.tile([128, CW], BF16, tag="g1", name="g1")
                    nc.scalar.activation(g1[:, :cw], ps[:, :cw], AF.Gelu,
                                         bias=b1T[:, h:h + 1])
                    g1s.append(g1)
                return g1s

            def patch_g2(ci, g1s):
                c0, cw = CHUNKS[ci]
                for dp in range(DT):
                    ps = pp.tile([128, 512], F32, tag="ps", name="ps")
                    for h in range(HT):
                        nc.tensor.matmul(
                            ps[:, :cw],
                            w2T[:, h * D + dp * 128:h * D + (dp + 1) * 128],
                            g1s[h][:, :cw],
                            start=(h == 0), stop=(h == HT - 1))
                    stg = op.tile([128, CW], BF16, tag="ostg", name="ostg")
                    nc.vector.tensor_copy(stg[:, :cw], ps[:, :cw])
                    # alternate store engines: the final two store chains and
                    # the end-of-kernel ring teardowns run concurrently
                    seng = nc.sync if dp % 2 == 0 else nc.gpsimd
                    seng.dma_start(poutT_d[dp][:, c0:c0 + cw],
                                        stg[:, :cw])

            g1s_c0 = patch_g1(0, xs_pre[0])
            g1s_c1 = patch_g1(1, xs_pre[1])

            # ---- phase A: grouped atom in-GEMM + gelu ----
            # cls tokens are host-permuted by src atom: group s occupies
            # columns [goff[s], goff[s+1]), so each token's hidden state is
            # computed only for its routed atom (1/5 the FLOPs of the dense
            # all-atom form).
            Gk = [gp.tile([128, NT], BF16, tag=f"g{k}", name=f"g{k}")
                  for k in range(KPA)]
            for s in range(NA):
                o0, o1 = goff[s], goff[s + 1]
                ns = o1 - o0
                if ns == 0:
                    continue
                for k in range(KPA):
                    ps = pp.tile([128, 512], F32, tag="ps", name="ps")
                    c0 = s * HSH + k * 128
                    for d in range(DT):
                        nc.tensor.matmul(
                            ps[:, :ns],
                            ainT[d][:, c0:c0 + 128],
                            clsT[:, d * NT + o0:d * NT + o1],
                            start=(d == 0), stop=(d == DT - 1))
                    nc.scalar.activation(Gk[k][:, o0:o1], ps[:, :ns],
                                         AF.Gelu,
                                         bias=ainbT[:, s * KPA + k:
                                                    s * KPA + k + 1])

            # ---- phase B: scale hidden by the gate weight (DVE) ----
            Hk = []
            for k in range(KPA):
                h = hp.tile([128, NT], BF16, tag=f"hid{k}", name=f"hid{k}")
                nc.vector.tensor_mul(h[:], Gk[k][:], wrep[:])
                Hk.append(h)

            patch_g2(0, g1s_c0)

            # ---- atom out-GEMM, grouped by dst atom ----
            # Each column range in dranges[a] holds tokens routed to atom a;
            # each range accumulates over the KPA h-shard tiles only.
            # PSUM zero regions are whole banks: the FIRST matmul into the
            # tile carries start=True (lazily zeroing the bank); every other
            # matmul accumulates — first touch of a pending byte zeroes it.
            nmm_out = sum(KPA * len(dranges[a]) for a in range(NA))
            for dp in range(DT):
                ps = pp.tile([128, 512], F32, tag="ps", name="ps")
                n = 0
                for a in range(NA):
                    for k in range(KPA):
                        for (r0, r1) in dranges[a]:
                            nc.tensor.matmul(
                                ps[:, r0:r1],
                                aoutT[a][:, k * D + dp * 128:
                                         k * D + (dp + 1) * 128],
                                Hk[k][:, r0:r1],
                                start=(n == 0), stop=(n == nmm_out - 1),
                                skip_group_check=True)
                            n += 1
                stg = op.tile([128, CW], BF16, tag="cstg", name="cstg")
                nc.vector.tensor_copy(stg[:, :NT], ps[:, :NT])
                ceng = nc.sync if dp % 2 == 0 else nc.gpsimd
                ceng.dma_start(cpartT_d[dp], stg[:, :NT])

            # ---- patch chunks 1..3 ----
            patch_g2(1, g1s_c1)
            xs_pre.append(load_x(2))
            patch_g2(2, patch_g1(2, xs_pre[2]))
            xs_pre.append(load_x(3))
            patch_g2(3, patch_g1(3, xs_pre[3]))

    nc.compile()
    return nc


def _sigmoid(x):
    out = np.empty_like(x)
    pos = x >= 0
    out[pos] = 1.0 / (1.0 + np.exp(-x[pos]))
    ex = np.exp(x[~pos])
    out[~pos] = ex / (1.0 + ex)
    return out


def kernel(x, patch_w1, patch_b1, patch_w2, patch_b2, gate_delta,
           atom_in_w, atom_in_b, atom_out_w, atom_out_b):
    x = np.asarray(x, dtype=np.float32)
    patch_w1 = np.asarray(patch_w1, dtype=np.float32)
    patch_b1 = np.asarray(patch_b1, dtype=np.float32)
    patch_w2 = np.asarray(patch_w2, dtype=np.float32)
    patch_b2 = np.asarray(patch_b2, dtype=np.float32)
    gate_delta = np.asarray(gate_delta, dtype=np.float32)
    atom_in_w = np.asarray(atom_in_w, dtype=np.float32)
    atom_in_b = np.asarray(atom_in_b, dtype=np.float32)
    atom_out_w = np.asarray(atom_out_w, dtype=np.float32)
    atom_out_b = np.asarray(atom_out_b, dtype=np.float32)

    bf = ml_dtypes.bfloat16

    # ---- host routing (tiny) ----
    cls3 = x[:, :NCLS, :]                                   # [B, 6, D]
    logits = np.einsum("bnd,nd->bn", cls3, gate_delta)      # [B, 6] f32
    choose_left = logits >= 0
    p_left = _sigmoid(logits)
    wgt = np.where(choose_left, p_left, 1.0 - p_left).astype(np.float32)
    keys = np.where(choose_left, LEFT_KEYS[None, :], RIGHT_KEYS[None, :])
    src = (keys // NA).reshape(-1)                          # [384]
    dst = (keys % NA).reshape(-1)
    wflat = wgt.reshape(-1)                                 # [384]

    # permute cls tokens by (src, dst) so each src atom's tokens are
    # contiguous and each dst atom's tokens are a few contiguous ranges
    order = np.lexsort((dst, src))
    inv_order = np.argsort(order)
    src_p, dst_p, wflat_p = src[order], dst[order], wflat[order]
    goff = tuple(int(np.searchsorted(src_p, s)) for s in range(NA + 1))
    dranges = []
    for a in range(NA):
        idx = np.flatnonzero(dst_p == a)
        ranges = []
        if idx.size:
            brk = np.flatnonzero(np.diff(idx) > 1)
            starts = np.concatenate(([0], brk + 1))
            ends = np.concatenate((brk, [idx.size - 1]))
            ranges = [(int(idx[s]), int(idx[e]) + 1)
                      for s, e in zip(starts, ends)]
        dranges.append(tuple(ranges))
    dranges = tuple(dranges)

    wrep_rep = np.ascontiguousarray(
        np.broadcast_to(wflat_p.reshape(1, NT), (128, NT))).astype(bf)

    # ---- replicated tensors (partition-major packed) ----
    # clsT[p, d*NT + t] = cls_permuted[t, d*128+p]
    clsT = np.ascontiguousarray(
        cls3.reshape(NT, D)[order].reshape(NT, DT, 128).transpose(2, 1, 0)
    ).reshape(128, DT * NT).astype(bf)
    # w1T[p, q, hh*768 + d*128 + m] = patch_w1[(q*3+hh)*128+m, d*128+p]
    # (hh-major within each piece so the first-needed h-tile is the piece's
    # first contiguous third)
    w1T = np.ascontiguousarray(
        patch_w1.reshape(NW1P, HPP, 128, DT, 128).transpose(4, 0, 1, 3, 2)
    ).reshape(128, NW1P, DT * HPP * 128).astype(bf)
    b1T = np.ascontiguousarray(patch_b1.reshape(HT, 128).T)
    # w2T[p, h*D + dp*128 + m] = patch_w2[dp*128+m, h*128+p]
    w2T = np.ascontiguousarray(
        patch_w2.reshape(DT, 128, HT, 128).transpose(3, 2, 0, 1)
    ).reshape(128, HT * D).astype(bf)

    # ---- per-core tensors ----
    patch = x[:, NCLS:, :].reshape(NCORES, TPC, D)
    # xT[p, ci*DT*CW + d*CW + t] = patch[c][ci*CW+t, d*128+p]
    xT_all = np.ascontiguousarray(
        patch.reshape(NCORES, NCH, CW, DT, 128).transpose(0, 4, 1, 3, 2)
    ).reshape(NCORES, 128, NCH * DT * CW).astype(bf)

    ainT_all, ainbT_all, aoutT_all = [], [], []
    for c in range(NCORES):
        hsl = slice(HSH * c, HSH * (c + 1))
        # ainT[d, p, a*HSH + k*128 + m] = atom_in_w[a, hsl0 + k*128+m, d*128+p]
        ainT = np.ascontiguousarray(
            atom_in_w[:, hsl, :].reshape(NA, KPA, 128, DT, 128)
            .transpose(3, 4, 0, 1, 2)).reshape(DT, 128, NA * HSH).astype(bf)
        ainT_all.append(ainT)
        ainbT_all.append(np.ascontiguousarray(
            atom_in_b[:, hsl].reshape(HLT, 128).T))
        # aoutT[a, p, k*D + dp*128 + m] = atom_out_w[a, dp*128+m, hsl0+k*128+p]
        aoutT = np.ascontiguousarray(
            atom_out_w[:, :, hsl].reshape(NA, DT, 128, KPA, 128)
            .transpose(0, 4, 3, 1, 2)).reshape(NA, 128, KPA * D).astype(bf)
        aoutT_all.append(aoutT)

    in_maps = []
    for c in range(NCORES):
        in_maps.append({
            "xT": xT_all[c], "w1T": w1T, "b1T": b1T, "w2T": w2T,
            "clsT": clsT, "ainT": ainT_all[c], "ainbT": ainbT_all[c],
            "aoutT": aoutT_all[c], "wrep": wrep_rep,
        })

    key = (goff, dranges)
    nc = _CACHE.get(key)
    if nc is None:
        nc = _build_program(goff, dranges)
        _CACHE[key] = nc

    res = run_bass_kernel_spmd(nc, in_maps, core_ids=list(range(NCORES)))
    global LAST_RESULTS
    LAST_RESULTS = res

    # ---- host gather ----
    patch_out = np.empty((B, P, D), dtype=np.float32)
    for c in range(NCORES):
        poutT = res.results[c]["poutT"].reshape(D, TPC).astype(np.float32)
        patch_out[BPC * c:BPC * (c + 1)] = (
            poutT.T + patch_b2[None, :]).reshape(BPC, P, D)

    cpart = np.zeros((D, NT), dtype=np.float32)
    for c in range(NCORES):
        cpart += res.results[c]["cpartT"].reshape(D, NT).astype(np.float32)
    cls_out = cpart.T[inv_order] + wflat[:, None] * atom_out_b[dst, :]
    cls_out = cls_out.reshape(B, NCLS, D)

    return np.concatenate([cls_out, patch_out], axis=1)


# revision 52
# speedup vs baseline: 1.0377x; 1.0080x over previous
"""Trainium2 Bass kernel for nn_Mlp_moe: dense patch-token MLP + top-1 gated
atom (expert) routing for 6 CLS task tokens.

Sharding over 8 NeuronCores:
  - Patch MLP: data-parallel over batch B=64 -> 8 batches (1568 patch tokens)
    per core. MLP weights replicated (SBUF-resident, bf16).
  - Atom/CLS part: hidden dim H=3072 sharded 8-way (384 per core); every core
    processes all 384 CLS tokens for all 5 atoms on its H-shard and emits a
    partial output summed on the host. Routing (gate logits/sigmoid/top-1
    masks) is computed on the host (it is O(B*6*D), negligible) and shipped
    as {0,1}/weight masks folded into the device compute.

Schedule (v2): the PE is warmed with dummy matmuls from program start (HAM
clock gate releases after ~3.4us of activity), while DMAs stream the first
working set (w1 piece 0 + x chunk 0) on need-ordered queues. w1 arrives in 8
pieces so GEMM1 starts as soon as the first 3 h-tiles + x0 land. Atom phases
run between chunk0's GEMM1 and GEMM2. Outputs stage through SBUF as bf16.

Device compute is bf16 (PSUM accumulation is fp32; erf-Gelu on ScalarE is
~exact); patch outputs are bf16, cls partials fp32.
"""

import numpy as np
import ml_dtypes

import concourse.bass as bass
import concourse.bacc as bacc
import concourse.mybir as mybir
from concourse import tile
from concourse.bass_utils import run_bass_kernel_spmd

NCORES = 8
B, NCLS, P, D, H = 64, 6, 196, 768, 3072
NA = 5
HSH = H // NCORES            # 384: per-core atom hidden shard
BPC = B // NCORES            # 8 batches per core
TPC = BPC * P                # 1568 patch tokens per core
NT = B * NCLS                # 384 cls tokens
DT = D // 128                # 6 d-tiles
HT = H // 128                # 24 h-tiles
HLT = NA * HSH // 128        # 15 atom h-shard tiles (a-major, 3 per atom)
KPA = HSH // 128             # 3 h-shard tiles per atom
CW = 392
NCH = 4
CHUNKS = [(i * CW, CW) for i in range(NCH)]
NW1P = 8                     # w1 DMA pieces (3 h-tiles each)
HPP = HT // NW1P             # 3 h-tiles per piece
NWARM = 30                   # PE warm-up dummy matmuls

LEFT_KEYS = np.array([3, 4, 8, 9, 13, 14], dtype=np.int64)
RIGHT_KEYS = np.array([15, 20, 16, 21, 17, 22], dtype=np.int64)

BF16 = mybir.dt.bfloat16
F32 = mybir.dt.float32
AF = mybir.ActivationFunctionType

_CACHE = {}
LAST_RESULTS = None  # BassKernelResults of the most recent run (for profiling)


def _build_program(goff, classes, doff):
    """goff: 6 cumulative offsets of the 5 src-atom token groups (cls tokens
    are host-permuted by (src, dst) so each atom's tokens are a contiguous
    column range). dranges[a]: list of (start, end) column ranges whose
    tokens route their output through atom a.
    """
    nc = bacc.Bacc(None, target_bir_lowering=False, debug=False,
                   num_devices=NCORES)

    # partition-major packed inputs (see host layouts in kernel())
    xT_d = nc.dram_tensor("xT", [128, NCH * DT * CW], BF16,
                          kind="ExternalInput")
    w1T_d = nc.dram_tensor("w1T", [128, NW1P, DT * HPP * 128], BF16,
                           kind="ExternalInput")
    b1T_d = nc.dram_tensor("b1T", [128, HT], F32, kind="ExternalInput")
    w2T_d = nc.dram_tensor("w2T", [128, HT * D], BF16, kind="ExternalInput")
    clsT_d = nc.dram_tensor("clsT", [128, DT * NT], BF16,
                            kind="ExternalInput")
    ainT_d = nc.dram_tensor("ainT", [DT, 128, NA * HSH], BF16,
                            kind="ExternalInput")
    ainbT_d = nc.dram_tensor("ainbT", [128, HLT], F32, kind="ExternalInput")
    aoutT_d = nc.dram_tensor("aoutT", [NA, 128, KPA * D], BF16,
                             kind="ExternalInput")
    wrep_d = nc.dram_tensor("wrep", [128, NT], BF16, kind="ExternalInput")
    poutT_d = nc.dram_tensor("poutT", [DT, 128, TPC], BF16,
                             kind="ExternalOutput")
    cpartT_d = nc.dram_tensor("cpartT", [DT, 128, NT], BF16,
                              kind="ExternalOutput")

    with tile.TileContext(nc) as tc:
        with (
            tc.tile_pool(name="w", bufs=1) as wp,
            tc.tile_pool(name="gat", bufs=1) as gp,
            tc.tile_pool(name="hida", bufs=1) as hp,
            tc.tile_pool(name="xin", bufs=2) as xp,
            tc.tile_pool(name="g1", bufs=48) as g1p,
            tc.tile_pool(name="ostg", bufs=4) as op,
            tc.tile_pool(name="ps", bufs=7, space="PSUM") as pp,
            tc.tile_pool(name="psw", bufs=1, space="PSUM") as pwp,
        ):
            # ---- PE warm-up: dummy matmuls from program start ----
            # The HAM clock gate holds the PE at 1.2 GHz until ~3.4us of
            # sustained activity; these dummies run while the first DMAs
            # stream in so the real matmuls start at 2.4 GHz.
            wdum = wp.tile([128, 512], BF16, tag="wdum", name="wdum")
            nc.vector.memset(wdum[:], 0.03125)
            pdum = pwp.tile([128, 256], F32, tag="pdum", name="pdum")
            for _ in range(NWARM):
                nc.tensor.matmul(pdum[:], wdum[:, :128], wdum[:, :256],
                                 start=True, stop=True)

            # ---- DMA issues: global need order striped over the 3 DMA
            # queues (sync/scalar/gpsimd) so the heads of all queues are
            # always the next-needed tensors and the shared DGE engine pool
            # serves the critical path first.
            def load_x(ci):
                xa = xp.tile([128, DT * CW], BF16, tag="x", name="x")
                nc.sync.dma_start(
                    xa[:], xT_d[:, ci * DT * CW:(ci + 1) * DT * CW])
                return xa

            w1T = [wp.tile([128, DT * HPP * 128], BF16, tag=f"w1{q}",
                           name=f"w1{q}") for q in range(NW1P)]
            b1T = wp.tile([128, HT], F32, tag="b1", name="b1")
            clsT = wp.tile([128, DT * NT], BF16, tag="cls", name="cls")
            w2T = wp.tile([128, HT * D], BF16, tag="w2", name="w2")
            ainbT = wp.tile([128, HLT], F32, tag="ainb", name="ainb")
            ainT = [wp.tile([128, NA * HSH], BF16, tag=f"ain{d}",
                            name=f"ain{d}") for d in range(DT)]
            wrep = wp.tile([128, NT], BF16, tag="wr", name="wr")
            aoutT = [wp.tile([128, KPA * D], BF16, tag=f"ao{a}",
                             name=f"ao{a}") for a in range(NA)]

            # wave 1: chunk0 GEMM1 working set. x0 and every w1 piece are
            # split in d-thirds round-robined over all three queues, so the
            # shared DGE pool delivers them in exact need order at full
            # aggregate bandwidth (piece k lands before the matmuls for
            # piece k-1 complete).
            # NOTE: an engine's next dma_start blocks until its previous
            # transfer completes, so ScalarE (which must run the gelus from
            # ~16us on) gets only the 3 earliest DMAs; sync/gpsimd (no
            # compute duties) carry everything else.
            # first-matmul critical set: w1 piece0's hh0 third rides ScalarE
            # first, x0 halves ride sync/gpsimd firsts -> first MM ~13us
            xa0 = xp.tile([128, DT * CW], BF16, tag="x", name="x")
            w1c = DT * HPP * 128
            nc.scalar.dma_start(w1T[0][:, :w1c // 3], w1T_d[:, 0, :w1c // 3])
            nc.sync.dma_start(xa0[:, :3 * CW], xT_d[:, :3 * CW])
            nc.gpsimd.dma_start(xa0[:, 3 * CW:DT * CW],
                                xT_d[:, 3 * CW:DT * CW])
            xs_pre = [xa0]
            nc.scalar.dma_start(b1T[:], b1T_d[:])
            nc.sync.dma_start(w1T[0][:, w1c // 3:2 * w1c // 3],
                              w1T_d[:, 0, w1c // 3:2 * w1c // 3])
            nc.gpsimd.dma_start(w1T[0][:, 2 * w1c // 3:],
                                w1T_d[:, 0, 2 * w1c // 3:])
            # pieces 1-7 in halves over three lanes: ScalarE's DMA lane is
            # idle from ~15us until its first gelu (~18us), so it carries two
            # early piece-halves (p1a, p3a), taking 0.6MB off the sync/gpsimd
            # streams whose delivery rate bounds when G1c0 can finish.
            lanes = {1: (nc.scalar, nc.gpsimd), 2: (nc.sync, nc.gpsimd),
                     3: (nc.scalar, nc.sync), 4: (nc.sync, nc.gpsimd),
                     5: (nc.gpsimd, nc.sync), 6: (nc.sync, nc.gpsimd),
                     7: (nc.gpsimd, nc.sync)}
            for p in range(1, NW1P):
                h1, h2 = lanes[p]
                h1.dma_start(w1T[p][:, :w1c // 2], w1T_d[:, p, :w1c // 2])
                h2.dma_start(w1T[p][:, w1c // 2:], w1T_d[:, p, w1c // 2:])
            # wave 2: chunk1 x (halves on both queues so it lands right as
            # G1c0's matmuls finish) + atom-in tensors
            xa1 = xp.tile([128, DT * CW], BF16, tag="x", name="x")
            nc.sync.dma_start(xa1[:, :3 * CW],
                              xT_d[:, DT * CW:DT * CW + 3 * CW])
            nc.gpsimd.dma_start(xa1[:, 3 * CW:],
                                xT_d[:, DT * CW + 3 * CW:2 * DT * CW])
            xs_pre.append(xa1)
            nc.gpsimd.dma_start(ainT[0][:], ainT_d[0])
            nc.sync.dma_start(clsT[:], clsT_d[:])
            nc.gpsimd.dma_start(ainT[1][:], ainT_d[1])
            nc.sync.dma_start(ainT[2][:], ainT_d[2])
            nc.gpsimd.dma_start(ainT[3][:], ainT_d[3])
            nc.sync.dma_start(ainT[4][:], ainT_d[4])
            nc.gpsimd.dma_start(ainT[5][:], ainT_d[5])
            nc.sync.dma_start(ainbT[:], ainbT_d[:])
            # wave 3: w2 + gate weights + atom-out tensors
            nc.gpsimd.dma_start(w2T[:, :12 * D], w2T_d[:, :12 * D])
            nc.sync.dma_start(w2T[:, 12 * D:], w2T_d[:, 12 * D:])
            nc.gpsimd.dma_start(wrep[:], wrep_d[:])
            nc.sync.dma_start(aoutT[0][:], aoutT_d[0])
            nc.gpsimd.dma_start(aoutT[1][:], aoutT_d[1])
            nc.sync.dma_start(aoutT[2][:], aoutT_d[2])
            nc.gpsimd.dma_start(aoutT[3][:], aoutT_d[3])
            nc.sync.dma_start(aoutT[4][:], aoutT_d[4])

            # ---- patch GEMM1 for one chunk (piece-gated on first chunk) ----
            def patch_g1(ci, xa):
                c0, cw = CHUNKS[ci]
                g1s = []
                for h in range(HT):
                    ps = pp.tile([128, 512], F32, tag="ps", name="ps")
                    q, hh = divmod(h, HPP)
                    for d in range(DT):
                        nc.tensor.matmul(
                            ps[:, :cw],
                            w1T[q][:, hh * DT * 128 + d * 128:
                                   hh * DT * 128 + (d + 1) * 128],
                            xa[:, d * CW:d * CW + cw],
                            start=(d == 0), stop=(d == DT - 1))
                    g1 = g1p.tile([128, CW], BF16, tag="g1", name="g1")
                    nc.scalar.activation(g1[:, :cw], ps[:, :cw], AF.Gelu,
                                         bias=b1T[:, h:h + 1])
                    g1s.append(g1)
                return g1s

            def patch_g2(ci, g1s):
                c0, cw = CHUNKS[ci]
                for dp in range(DT):
                    ps = pp.tile([128, 512], F32, tag="ps", name="ps")
                    for h in range(HT):
                        nc.tensor.matmul(
                            ps[:, :cw],
                            w2T[:, h * D + dp * 128:h * D + (dp + 1) * 128],
                            g1s[h][:, :cw],
                            start=(h == 0), stop=(h == HT - 1))
                    stg = op.tile([128, CW], BF16, tag="ostg", name="ostg")
                    nc.vector.tensor_copy(stg[:, :cw], ps[:, :cw])
                    # alternate store engines: the final two store chains and
                    # the end-of-kernel ring teardowns run concurrently
                    seng = nc.sync if dp % 2 == 0 else nc.gpsimd
                    seng.dma_start(poutT_d[dp][:, c0:c0 + cw],
                                        stg[:, :cw])

            g1s_c0 = patch_g1(0, xs_pre[0])
            g1s_c1 = patch_g1(1, xs_pre[1])

            # ---- phase A: grouped atom in-GEMM + gelu ----
            # cls tokens are host-permuted by src atom: group s occupies
            # columns [goff[s], goff[s+1]), so each token's hidden state is
            # computed only for its routed atom (1/5 the FLOPs of the dense
            # all-atom form).
            Gk = [gp.tile([128, NT], BF16, tag=f"g{k}", name=f"g{k}")
                  for k in range(KPA)]
            for s in range(NA):
                o0, o1 = goff[s], goff[s + 1]
                ns = o1 - o0
                if ns == 0:
                    continue
                for k in range(KPA):
                    ps = pp.tile([128, 512], F32, tag="ps", name="ps")
                    c0 = s * HSH + k * 128
                    for d in range(DT):
                        nc.tensor.matmul(
                            ps[:, :ns],
                            ainT[d][:, c0:c0 + 128],
                            clsT[:, d * NT + o0:d * NT + o1],
                            start=(d == 0), stop=(d == DT - 1))
                    nc.scalar.activation(Gk[k][:, o0:o1], ps[:, :ns],
                                         AF.Gelu,
                                         bias=ainbT[:, s * KPA + k:
                                                    s * KPA + k + 1])

            # ---- phase B: scale hidden by the gate weight (DVE), and
            # simultaneously re-permute columns into dst-major order so the
            # out-GEMM needs only one contiguous range per dst atom.
            # classes: (sd_off, ds_off, n) block moves; doff: dst offsets.
            Hk = []
            for k in range(KPA):
                h = hp.tile([128, NT], BF16, tag=f"hid{k}", name=f"hid{k}")
                for (so, do, n) in classes:
                    nc.vector.tensor_mul(h[:, do:do + n],
                                         Gk[k][:, so:so + n],
                                         wrep[:, so:so + n])
                Hk.append(h)

            patch_g2(0, g1s_c0)

            # ---- atom out-GEMM, one contiguous token range per dst atom.
            # PSUM zero regions are whole banks: the FIRST matmul into the
            # tile carries start=True (lazily zeroing the bank); every other
            # matmul accumulates — first touch of a pending byte zeroes it.
            nmm_out = sum(KPA for a in range(NA) if doff[a + 1] > doff[a])
            for dp in range(DT):
                ps = pp.tile([128, 512], F32, tag="ps", name="ps")
                n = 0
                for a in range(NA):
                    r0, r1 = doff[a], doff[a + 1]
                    if r0 == r1:
                        continue
                    for k in range(KPA):
                        nc.tensor.matmul(
                            ps[:, r0:r1],
                            aoutT[a][:, k * D + dp * 128:
                                     k * D + (dp + 1) * 128],
                            Hk[k][:, r0:r1],
                            start=(n == 0), stop=(n == nmm_out - 1),
                            skip_group_check=True)
                        n += 1
                stg = op.tile([128, CW], BF16, tag="cstg", name="cstg")
                nc.vector.tensor_copy(stg[:, :NT], ps[:, :NT])
                ceng = nc.sync if dp % 2 == 0 else nc.gpsimd
                ceng.dma_start(cpartT_d[dp], stg[:, :NT])

            # ---- patch chunks 1..3 ----
            patch_g2(1, g1s_c1)
            xs_pre.append(load_x(2))
            patch_g2(2, patch_g1(2, xs_pre[2]))
            xs_pre.append(load_x(3))
            patch_g2(3, patch_g1(3, xs_pre[3]))

    nc.compile()
    return nc


def _sigmoid(x):
    out = np.empty_like(x)
    pos = x >= 0
    out[pos] = 1.0 / (1.0 + np.exp(-x[pos]))
    ex = np.exp(x[~pos])
    out[~pos] = ex / (1.0 + ex)
    return out


def kernel(x, patch_w1, patch_b1, patch_w2, patch_b2, gate_delta,
           atom_in_w, atom_in_b, atom_out_w, atom_out_b):
    x = np.asarray(x, dtype=np.float32)
    patch_w1 = np.asarray(patch_w1, dtype=np.float32)
    patch_b1 = np.asarray(patch_b1, dtype=np.float32)
    patch_w2 = np.asarray(patch_w2, dtype=np.float32)
    patch_b2 = np.asarray(patch_b2, dtype=np.float32)
    gate_delta = np.asarray(gate_delta, dtype=np.float32)
    atom_in_w = np.asarray(atom_in_w, dtype=np.float32)
    atom_in_b = np.asarray(atom_in_b, dtype=np.float32)
    atom_out_w = np.asarray(atom_out_w, dtype=np.float32)
    atom_out_b = np.asarray(atom_out_b, dtype=np.float32)

    bf = ml_dtypes.bfloat16

    # ---- host routing (tiny) ----
    cls3 = x[:, :NCLS, :]                                   # [B, 6, D]
    logits = np.einsum("bnd,nd->bn", cls3, gate_delta)      # [B, 6] f32
    choose_left = logits >= 0
    p_left = _sigmoid(logits)
    wgt = np.where(choose_left, p_left, 1.0 - p_left).astype(np.float32)
    keys = np.where(choose_left, LEFT_KEYS[None, :], RIGHT_KEYS[None, :])
    src = (keys // NA).reshape(-1)                          # [384]
    dst = (keys % NA).reshape(-1)
    wflat = wgt.reshape(-1)                                 # [384]

    # permute cls tokens by (src, dst) so each src atom's tokens are
    # contiguous and each dst atom's tokens are a few contiguous ranges
    order = np.lexsort((dst, src))       # src-major: in-GEMM / gelu order
    src_p, dst_p, wflat_p = src[order], dst[order], wflat[order]
    goff = tuple(int(np.searchsorted(src_p, s)) for s in range(NA + 1))
    order_ds = np.lexsort((src, dst))    # dst-major: out-GEMM order
    dst_ds, src_ds = dst[order_ds], src[order_ds]
    doff = tuple(int(np.searchsorted(dst_ds, a)) for a in range(NA + 1))
    inv_ds = np.argsort(order_ds)
    # (src,dst) classes are contiguous in BOTH orders (stable lexsort keeps
    # within-class token order), so the reorder is a set of block moves
    classes = []
    for s in range(NA):
        for a in range(NA):
            n = int(((src == s) & (dst == a)).sum())
            if n == 0:
                continue
            so = int(np.flatnonzero((src_p == s) & (dst_p == a))[0])
            do = int(np.flatnonzero((src_ds == s) & (dst_ds == a))[0])
            classes.append((so, do, n))
    classes = tuple(classes)

    wrep_rep = np.ascontiguousarray(
        np.broadcast_to(wflat_p.reshape(1, NT), (128, NT))).astype(bf)

    # ---- replicated tensors (partition-major packed) ----
    # clsT[p, d*NT + t] = cls_permuted[t, d*128+p]
    clsT = np.ascontiguousarray(
        cls3.reshape(NT, D)[order].reshape(NT, DT, 128).transpose(2, 1, 0)
    ).reshape(128, DT * NT).astype(bf)
    # w1T[p, q, hh*768 + d*128 + m] = patch_w1[(q*3+hh)*128+m, d*128+p]
    # (hh-major within each piece so the first-needed h-tile is the piece's
    # first contiguous third)
    w1T = np.ascontiguousarray(
        patch_w1.reshape(NW1P, HPP, 128, DT, 128).transpose(4, 0, 1, 3, 2)
    ).reshape(128, NW1P, DT * HPP * 128).astype(bf)
    b1T = np.ascontiguousarray(patch_b1.reshape(HT, 128).T)
    # w2T[p, h*D + dp*128 + m] = patch_w2[dp*128+m, h*128+p]
    w2T = np.ascontiguousarray(
        patch_w2.reshape(DT, 128, HT, 128).transpose(3, 2, 0, 1)
    ).reshape(128, HT * D).astype(bf)

    # ---- per-core tensors ----
    patch = x[:, NCLS:, :].reshape(NCORES, TPC, D)
    # xT[p, ci*DT*CW + d*CW + t] = patch[c][ci*CW+t, d*128+p]
    xT_all = np.ascontiguousarray(
        patch.reshape(NCORES, NCH, CW, DT, 128).transpose(0, 4, 1, 3, 2)
    ).reshape(NCORES, 128, NCH * DT * CW).astype(bf)

    ainT_all, ainbT_all, aoutT_all = [], [], []
    for c in range(NCORES):
        hsl = slice(HSH * c, HSH * (c + 1))
        # ainT[d, p, a*HSH + k*128 + m] = atom_in_w[a, hsl0 + k*128+m, d*128+p]
        ainT = np.ascontiguousarray(
            atom_in_w[:, hsl, :].reshape(NA, KPA, 128, DT, 128)
            .transpose(3, 4, 0, 1, 2)).reshape(DT, 128, NA * HSH).astype(bf)
        ainT_all.append(ainT)
        ainbT_all.append(np.ascontiguousarray(
            atom_in_b[:, hsl].reshape(HLT, 128).T))
        # aoutT[a, p, k*D + dp*128 + m] = atom_out_w[a, dp*128+m, hsl0+k*128+p]
        aoutT = np.ascontiguousarray(
            atom_out_w[:, :, hsl].reshape(NA, DT, 128, KPA, 128)
            .transpose(0, 4, 3, 1, 2)).reshape(NA, 128, KPA * D).astype(bf)
        aoutT_all.append(aoutT)

    in_maps = []
    for c in range(NCORES):
        in_maps.append({
            "xT": xT_all[c], "w1T": w1T, "b1T": b1T, "w2T": w2T,
            "clsT": clsT, "ainT": ainT_all[c], "ainbT": ainbT_all[c],
            "aoutT": aoutT_all[c], "wrep": wrep_rep,
        })

    key = (goff, classes, doff)
    nc = _CACHE.get(key)
    if nc is None:
        nc = _build_program(goff, classes, doff)
        _CACHE[key] = nc

    res = run_bass_kernel_spmd(nc, in_maps, core_ids=list(range(NCORES)))
    global LAST_RESULTS
    LAST_RESULTS = res

    # ---- host gather ----
    patch_out = np.empty((B, P, D), dtype=np.float32)
    for c in range(NCORES):
        poutT = res.results[c]["poutT"].reshape(D, TPC).astype(np.float32)
        patch_out[BPC * c:BPC * (c + 1)] = (
            poutT.T + patch_b2[None, :]).reshape(BPC, P, D)

    cpart = np.zeros((D, NT), dtype=np.float32)
    for c in range(NCORES):
        cpart += res.results[c]["cpartT"].reshape(D, NT).astype(np.float32)
    cls_out = cpart.T[inv_ds] + wflat[:, None] * atom_out_b[dst, :]
    cls_out = cls_out.reshape(B, NCLS, D)

    return np.concatenate([cls_out, patch_out], axis=1)
